# revision 1
# baseline (speedup 1.0000x reference)
"""CSSM TinyViT block on 8 TRN2 NeuronCores — fp8 DoubleRow version.

Strategy
--------
Data-parallel over batch: B=16 -> 2 samples (2048 tokens) per core, processed
as 4 groups of 512 tokens in an offset-by-one-group software pipeline.

All channel-mixing matmuls run fp8(e4m3) DoubleRow (0.5 cyc/row on the PE):
weights are scaled by WS=64 host-side and the 1/WS dequant is folded into the
per-element epilogues.  Biases are folded into the matmuls via a constant-1
padding channel whose weight row is bias*WS.

The scan h' <- g*(A^T h') + m2 (flipped sign, m2=(g-1)(u+bu)) runs on the
p-recurrence p_{t+1} = [A;A]^T [g*p_t ; m2], which needs only ONE elementwise
op per step (y = (p*0.5/WS)*(tanh+1), a scalar_tensor_tensor) instead of a
multiply+add.  The gate is kept as thp=tanh+1 / thm=tanh-1 so both the scan
multiply and m2 are single fused STT ops.

LayerNorm: bn_stats/bn_aggr token-major on DVE; rstd via the rsqrt bit hack +
one Newton step (no ACT Sqrt -> the single act table tanh/gelu/identity/copy
is never swapped).  Transposes run bf16 on the PE; the PSUM->SBUF copies
quantize to fp8 (round-to-nearest, verified on HW).

PSUM: scan p [128,3,512] (3 banks, bufs=1) + pst transposes (3x1 bank) +
general pool (2 banks) = 8 banks exactly.
"""
import json
import os
import types

import numpy as np

import concourse.bass as bass
import concourse.mybir as mybir
from concourse.tile import TileContext
from concourse.bass_utils import run_bass_kernel_spmd

F32 = mybir.dt.float32
FP8 = mybir.dt.float8e4
BF16 = mybir.dt.bfloat16
I32 = mybir.dt.int32
AF = mybir.ActivationFunctionType
OP = mybir.AluOpType
DR = mybir.MatmulPerfMode.DoubleRow

B, H, W, C, T = 16, 32, 32, 384, 8
HID = 4 * C
EPS = 1e-6
NCORES = 8
BSH = B // NCORES
NTOK = BSH * H * W             # 2048 tokens per core
GTOK = 512                     # tokens per group
NG = NTOK // GTOK              # 4 groups
TPG = GTOK // 128              # 4 token-tiles per group
KT = C // 128                  # 3 channel tiles
MH = HID // 128                # 12 hidden tiles
WS = 64.0                      # fp8 weight scale
SH = 0.5 / WS                  # scan/gate dequant incl. the tanh 0.5
IWS = 1.0 / WS
MAGIC = 0x5F3759DF

CFG = {
    "scan_route": os.environ.get("K2_SCAN", "route"),   # route | dve3
    "apply1": os.environ.get("K2_APPLY1", "pool"),      # act | pool
    "thp_eng": os.environ.get("K2_THP", "pool"),        # dve | pool
    "mout": os.environ.get("K2_MOUT", "pp"),            # dp | pp | dd
    "pool_off": os.environ.get("K2_POOLOFF", "1") == "1",
    "thp_f32": os.environ.get("K2_THPF32", "0") == "1",
}
if CFG["pool_off"]:
    CFG["scan_route"] = "dve3"
    CFG["apply1"] = "act"
    CFG["thp_eng"] = "dve"
    CFG["mout"] = "dd"


# ---------------------------------------------------------------- bir fix --
# This container's walrus rejects instructions whose sync-wait list exceeds
# the opcode's wait slots (an SP Drain has none free).  Move excess waits
# onto EventSemaphore instructions inserted before the instruction on the
# same engine queue; waits still happen-before, so semantics are unchanged.
_WAIT_LIMITS = {"Drain": 0}
_WAIT_DEFAULT = 1


def _fix_bir_json(bj: bytes) -> bytes:
    bir = json.loads(bj)
    counter = [0]

    def fix_blocks(blocks):
        for b in blocks:
            insts = b.get("instructions")
            if insts:
                new = []
                for inst in insts:
                    si = inst.get("sync_info")
                    waits = (si or {}).get("on_wait") or []
                    limit = _WAIT_LIMITS.get(inst.get("opcode"), _WAIT_DEFAULT)
                    if len(waits) > limit:
                        n_extra = len(waits) - limit
                        extra, keep = waits[:n_extra], waits[n_extra:]
                        for wv in extra:
                            counter[0] += 1
                            new.append({
                                "name": f"I-wfix-{counter[0]}",
                                "opcode": "EventSemaphore",
                                "engine": inst["engine"],
                                "ins": [],
                                "outs": [],
                                "sync_info": {"on_update": [], "on_wait": [wv]},
                                "debug": inst.get("debug", 0),
                            })
                        si["on_wait"] = keep
                    new.append(inst)
                b["instructions"] = new
            fix_blocks(b.get("blocks") or [])

    for fn in bir.get("functions", []):
        fix_blocks(fn.get("blocks") or [])
    return json.dumps(bir).encode()


def _patch_nc(nc):
    orig = nc.to_json_bytes

    def to_json_bytes(self):
        return _fix_bir_json(orig())

    nc.to_json_bytes = types.MethodType(to_json_bytes, nc)
    return nc


# ----------------------------------------------------------- device build --
def build_nc(repeat=1):
    nc = bass.Bass()

    x_in = nc.declare_dram_parameter("x", [NTOK, C], F32, isOutput=False)
    wu_d = nc.declare_dram_parameter("wu", [128, 4, C], FP8, isOutput=False)
    wg_d = nc.declare_dram_parameter("wg", [128, 4, C], FP8, isOutput=False)
    a_d = nc.declare_dram_parameter("a", [128, 6, C], FP8, isOutput=False)
    w1_d = nc.declare_dram_parameter("w1", [128, 4, HID], FP8, isOutput=False)
    w2_d = nc.declare_dram_parameter("w2", [128, MH, C], FP8, isOutput=False)
    ones_d = nc.declare_dram_parameter("ones", [1, 128], FP8, isOutput=False)
    b2_d = nc.declare_dram_parameter("b2", [1, C], FP8, isOutput=False)
    pad_d = nc.declare_dram_parameter("pad", [128, GTOK], FP8, isOutput=False)
    eye_d = nc.declare_dram_parameter("eye", [128, 128], BF16, isOutput=False)
    out_d = nc.declare_dram_parameter("out", [NTOK, C], F32, isOutput=True)

    N = NG * repeat

    with TileContext(nc) as tc:
        with (
            tc.tile_pool(name="wp", bufs=1) as wp,
            tc.tile_pool(name="gp", bufs=2) as gp,
            tc.tile_pool(name="gp3", bufs=3) as gp3,
            tc.tile_pool(name="xp", bufs=4) as xp,
            tc.tile_pool(name="tp", bufs=2) as tp,
            tc.tile_pool(name="sp", bufs=2) as sp,
            tc.tile_pool(name="ps", bufs=2, space="PSUM") as ps,
            tc.tile_pool(name="scanp", bufs=1, space="PSUM") as scanp,
            tc.tile_pool(name="pstp", bufs=3, space="PSUM") as pstp,
        ):
            # ---- weights / constants (loaded once) ----
            wu_t = wp.tile([128, 4, C], FP8, tag="wu")
            wg_t = wp.tile([128, 4, C], FP8, tag="wg")
            a_t = wp.tile([128, 6, C], FP8, tag="a")
            w1_t = wp.tile([128, 4, HID], FP8, tag="w1")
            w2_t = wp.tile([128, MH, C], FP8, tag="w2")
            ones_t = wp.tile([1, 128], FP8, tag="ones")
            b2_t = wp.tile([1, C], FP8, tag="b2")
            eye_t = wp.tile([128, 128], BF16, tag="eye")
            nc.sync.dma_start(out=eye_t, in_=eye_d[:, :])
            nc.sync.dma_start(out=wu_t, in_=wu_d[:, :, :])
            nc.sync.dma_start(out=wg_t, in_=wg_d[:, :, :])
            nc.sync.dma_start(out=a_t, in_=a_d[:, :, :])

            def load_late_weights():
                nc.sync.dma_start(out=w1_t, in_=w1_d[:, :, :])
                nc.sync.dma_start(out=w2_t, in_=w2_d[:, :, :])
                nc.sync.dma_start(out=ones_t, in_=ones_d[:, :])
                nc.sync.dma_start(out=b2_t, in_=b2_d[:, :])

            insts = [dict(idx=i) for i in range(N)]

            # ------------- phase A: load + LN1 stats/apply + transpose ----
            def a_load(st, it):
                if it == 0:
                    st["x_tm"] = xp.tile([128, TPG, C], F32, tag="xtm",
                                         name=f"xtm{st['idx']}")
                    st["mvb"] = sp.tile([128, TPG, 2], F32, tag="mvb",
                                        name=f"mvb{st['idx']}")
                row0 = (st["idx"] % NG) * GTOK + it * 128
                nc.sync.dma_start(out=st["x_tm"][:, it, :],
                                  in_=x_in[row0:row0 + 128, :])
                mv6 = sp.tile([128, 6], F32, tag="mv6", bufs=2)
                nc.vector.bn_stats(out=mv6, in_=st["x_tm"][:, it, :])
                nc.vector.bn_aggr(out=st["mvb"][:, it, :], in_=mv6)

            def _rsqrt(mvb, tag):
                """r1 = rsqrt(var+eps), mnr = -mean*r1, each [128,TPG,1]."""
                S = [128, TPG, 1]
                vpe = sp.tile(S, F32, tag=f"vpe{tag}")
                nc.vector.tensor_scalar(out=vpe, in0=mvb[:, :, 1:2],
                                        scalar1=EPS, scalar2=None, op0=OP.add)
                i1 = sp.tile(S, I32, tag=f"i1{tag}")
                nc.vector.tensor_scalar(out=i1, in0=vpe.bitcast(I32),
                                        scalar1=1, scalar2=None,
                                        op0=OP.logical_shift_right)
                i2 = sp.tile(S, I32, tag=f"i2{tag}")
                nc.vector.tensor_scalar(out=i2, in0=i1, scalar1=MAGIC,
                                        scalar2=-1, op0=OP.subtract,
                                        op1=OP.mult)
                r0 = i2.bitcast(F32)
                t = sp.tile(S, F32, tag=f"t{tag}")
                nc.vector.tensor_mul(out=t, in0=vpe, in1=r0)
                nc.vector.tensor_mul(out=t, in0=t, in1=r0)
                nc.vector.tensor_scalar(out=t, in0=t, scalar1=-0.5,
                                        scalar2=1.5, op0=OP.mult, op1=OP.add)
                rA = sp.tile(S, F32, tag=f"rA{tag}")
                nc.vector.tensor_mul(out=rA, in0=t, in1=r0)
                t2 = sp.tile(S, F32, tag=f"t2{tag}")
                nc.vector.tensor_mul(out=t2, in0=vpe, in1=rA)
                nc.vector.tensor_mul(out=t2, in0=t2, in1=rA)
                nc.vector.tensor_scalar(out=t2, in0=t2, scalar1=-0.5,
                                        scalar2=1.5, op0=OP.mult, op1=OP.add)
                r1 = sp.tile(S, F32, tag=f"r1{tag}")
                nc.vector.tensor_mul(out=r1, in0=t2, in1=rA)
                nr = sp.tile(S, F32, tag=f"nr{tag}")
                nc.vector.tensor_scalar(out=nr, in0=r1, scalar1=-1.0,
                                        scalar2=None, op0=OP.mult)
                mnr = sp.tile(S, F32, tag=f"mnr{tag}")
                nc.vector.tensor_mul(out=mnr, in0=mvb[:, :, 0:1], in1=nr)
                return r1, mnr

            def a_stats(st):
                st["r1"], st["mnr"] = _rsqrt(st["mvb"], "a")

            def a_apply(st, it):
                xn = tp.tile([128, C], BF16, tag="xntm", bufs=2)
                if CFG["apply1"] == "act":
                    nc.scalar.activation(out=xn, in_=st["x_tm"][:, it, :],
                                         func=AF.Identity,
                                         scale=st["r1"][:, it, :],
                                         bias=st["mnr"][:, it, :])
                else:
                    nc.gpsimd.tensor_scalar(out=xn, in0=st["x_tm"][:, it, :],
                                            scalar1=st["mvb"][:, it, 0:1],
                                            scalar2=st["r1"][:, it, :],
                                            op0=OP.subtract, op1=OP.mult)
                pair, half = divmod(it, 2)
                if half == 0:
                    st[f"pst_a{pair}"] = pstp.tile([128, KT, 2, 128], BF16,
                                                   tag="pst", name="psta")
                pt = st[f"pst_a{pair}"]
                for c in range(KT):
                    nc.tensor.transpose(pt[:, c, half, :],
                                        xn[:, c * 128:(c + 1) * 128], eye_t)

            def a_copy(st, pair, eng):
                if pair == 0:
                    st["xn_cm"] = gp.tile([128, 4, GTOK], FP8, tag="xncm",
                                          name=f"xncm{st['idx']}")
                    nc.sync.dma_start(out=st["xn_cm"][:, 3, :],
                                      in_=pad_d[:, :])
                src = st[f"pst_a{pair}"]
                dst = st["xn_cm"][:, 0:KT, pair * 256:(pair + 1) * 256]
                dst = dst.rearrange("p c (i q) -> p c i q", i=2)
                if eng == "act":
                    nc.scalar.activation(out=dst, in_=src, func=AF.Copy)
                else:
                    nc.vector.tensor_copy(dst, src)

            # ------------- phase B: u/g projections, gate, m2 -------------
            def b_proj(st, m):
                if m == 0:
                    st["ybuf"] = gp.tile([128, 6, GTOK], FP8, tag="ybuf",
                                         name=f"ybuf{st['idx']}")
                    THDT = F32 if CFG["thp_f32"] else BF16
                    st["thp"] = gp.tile([128, KT, GTOK], THDT, tag="thp",
                                        name=f"thp{st['idx']}")
                    st["thm"] = gp.tile([128, KT, GTOK], THDT, tag="thm",
                                        name=f"thm{st['idx']}")
                    st["m2b"] = gp.tile([128, KT, GTOK], BF16, tag="m2b",
                                        name=f"m2b{st['idx']}")
                msl = slice(m * 128, (m + 1) * 128)
                xn_cm = st["xn_cm"]
                psu = ps.tile([128, GTOK], F32, tag="ps", name="psu")
                for j in range(2):
                    nc.tensor.matmul(psu, wu_t[:, 2 * j:2 * j + 2, msl],
                                     xn_cm[:, 2 * j:2 * j + 2, :],
                                     start=(j == 0), stop=(j == 1),
                                     perf_mode=DR)
                psg = ps.tile([128, GTOK], F32, tag="ps", name="psg")
                for j in range(2):
                    nc.tensor.matmul(psg, wg_t[:, 2 * j:2 * j + 2, msl],
                                     xn_cm[:, 2 * j:2 * j + 2, :],
                                     start=(j == 0), stop=(j == 1),
                                     perf_mode=DR)
                thraw = tp.tile([128, GTOK],
                                F32 if CFG["thp_f32"] else BF16,
                                tag="thraw", bufs=2)
                nc.scalar.activation(out=thraw, in_=psg, func=AF.Tanh,
                                     scale=SH)
                te = nc.vector if CFG["thp_eng"] == "dve" else nc.gpsimd
                te.tensor_scalar(out=st["thp"][:, m, :], in0=thraw,
                                 scalar1=1.0, scalar2=None, op0=OP.add)
                tm = nc.vector if CFG["pool_off"] else nc.gpsimd
                tm.tensor_scalar(out=st["thm"][:, m, :], in0=thraw,
                                 scalar1=-1.0, scalar2=None, op0=OP.add)
                nc.vector.scalar_tensor_tensor(
                    out=st["m2b"][:, m, :], in0=psu, scalar=SH,
                    in1=st["thm"][:, m, :], op0=OP.mult, op1=OP.mult)
                nc.scalar.activation(out=st["ybuf"][:, KT + m, :],
                                     in_=st["m2b"][:, m, :], func=AF.Copy)

            # ------------- scan ------------------------------------------
            def s_p1(st, m):
                if m == 0:
                    st["p"] = scanp.tile([128, KT, GTOK], F32, tag="p",
                                         name="scp")
                msl = slice(m * 128, (m + 1) * 128)
                y = st["ybuf"]
                nc.tensor.matmul(st["p"][:, m, :], a_t[:, 3:5, msl],
                                 y[:, 3:5, :], start=True, stop=False,
                                 perf_mode=DR)
                nc.tensor.matmul(st["p"][:, m, :], a_t[:, 5, msl],
                                 y[:, 5, :], start=False, stop=True)

            def s_round_route(st):
                """ACT+Pool route for bank 2 of the NEXT y — emitted right
                after the matmuls producing p so the ACT queue reaches it
                before any fill."""
                return

            def s_round_y(st):
                p, y, thp = st["p"], st["ybuf"], st["thp"]
                nc.vector.scalar_tensor_tensor(
                    out=y[:, 0:KT, :], in0=p, scalar=SH, in1=thp,
                    op0=OP.mult, op1=OP.mult)

            def s_round_mm(st):
                p, y = st["p"], st["ybuf"]
                for m in range(KT):
                    msl = slice(m * 128, (m + 1) * 128)
                    for j in range(KT):
                        nc.tensor.matmul(p[:, m, :],
                                         a_t[:, 2 * j:2 * j + 2, msl],
                                         y[:, 2 * j:2 * j + 2, :],
                                         start=(j == 0), stop=(j == KT - 1),
                                         perf_mode=DR)

            def s_final_route(st):
                t1 = tp.tile([128, KT, GTOK], BF16, tag="t1", bufs=2,
                             name="t1f")
                st["t1"] = t1

            def s_final(st):
                p, y, thp, t1 = st["p"], st["ybuf"], st["thp"], st["t1"]
                nc.vector.scalar_tensor_tensor(
                    out=t1, in0=p, scalar=SH, in1=thp,
                    op0=OP.mult, op1=OP.mult)
                h = gp.tile([128, KT, GTOK], BF16, tag="hcm",
                            name=f"hcm{st['idx']}")
                st["h_cm"] = h
                m2b = st["m2b"]
                if CFG["pool_off"]:
                    nc.vector.tensor_add(out=h, in0=t1, in1=m2b)
                else:
                    nc.vector.tensor_add(out=h[:, 0:1, :], in0=t1[:, 0:1, :],
                                         in1=m2b[:, 0:1, :])
                    nc.gpsimd.tensor_add(out=h[:, 1:3, :],
                                         in0=t1[:, 1:3, :],
                                         in1=m2b[:, 1:3, :])

            # ------------- residual 1 ------------------------------------
            def r_pair(st, pair, eng):
                if pair == 0:
                    st["x2_tm"] = gp3.tile([128, TPG, C], F32, tag="x2tm",
                                          name=f"x2tm{st['idx']}")
                pst = pstp.tile([128, KT, 2, 128], BF16, tag="pst",
                                name="pstr")
                for half in range(2):
                    it = pair * 2 + half
                    tsl = slice(it * 128, (it + 1) * 128)
                    for c in range(KT):
                        nc.tensor.transpose(pst[:, c, half, :],
                                            st["h_cm"][:, c, tsl], eye_t)
                dst = st["x2_tm"][:, pair * 2:pair * 2 + 2, :]
                dst = dst.rearrange("p i (c q) -> p i c q", c=KT)
                src = st["x_tm"][:, pair * 2:pair * 2 + 2, :]
                src = src.rearrange("p i (c q) -> p i c q", c=KT)
                hsrc = pst.rearrange("p c i q -> p i c q")
                if eng == "dve" or CFG["pool_off"]:
                    nc.vector.tensor_sub(out=dst, in0=src, in1=hsrc)
                else:
                    hb = tp.tile([128, 2, KT, 128], BF16, tag="hb", bufs=2)
                    nc.scalar.activation(out=hb, in_=hsrc, func=AF.Copy)
                    nc.gpsimd.tensor_sub(out=dst, in0=src, in1=hb)

            # ------------- norm 2 ----------------------------------------
            def n2_load(st, it):
                if it == 0:
                    st["mvb2"] = sp.tile([128, TPG, 2], F32, tag="mvb2",
                                         name=f"mvb2{st['idx']}")
                mv6 = sp.tile([128, 6], F32, tag="mv6", bufs=2)
                nc.vector.bn_stats(out=mv6, in_=st["x2_tm"][:, it, :])
                nc.vector.bn_aggr(out=st["mvb2"][:, it, :], in_=mv6)

            def n2_stats(st):
                st["r2"], st["mnr2"] = _rsqrt(st["mvb2"], "b")

            def n2_apply(st, it):
                xn2 = tp.tile([128, C], BF16, tag="xn2tm", bufs=2)
                if CFG["pool_off"]:
                    nc.scalar.activation(out=xn2, in_=st["x2_tm"][:, it, :],
                                         func=AF.Identity,
                                         scale=st["r2"][:, it, :],
                                         bias=st["mnr2"][:, it, :])
                else:
                    nc.gpsimd.tensor_scalar(out=xn2,
                                            in0=st["x2_tm"][:, it, :],
                                            scalar1=st["mvb2"][:, it, 0:1],
                                            scalar2=st["r2"][:, it, :],
                                            op0=OP.subtract, op1=OP.mult)
                pair, half = divmod(it, 2)
                if half == 0:
                    st[f"pst_n{pair}"] = pstp.tile([128, KT, 2, 128], BF16,
                                                   tag="pst", name="pstn")
                pt = st[f"pst_n{pair}"]
                for c in range(KT):
                    nc.tensor.transpose(pt[:, c, half, :],
                                        xn2[:, c * 128:(c + 1) * 128], eye_t)

            def n2_copy(st, pair, eng):
                if pair == 0:
                    st["xn2_cm"] = gp3.tile([128, 4, GTOK], FP8, tag="xn2cm",
                                           name=f"xn2cm{st['idx']}")
                    nc.sync.dma_start(out=st["xn2_cm"][:, 3, :],
                                      in_=pad_d[:, :])
                src = st[f"pst_n{pair}"]
                dst = st["xn2_cm"][:, 0:KT, pair * 256:(pair + 1) * 256]
                dst = dst.rearrange("p c (i q) -> p c i q", i=2)
                if eng == "act":
                    nc.scalar.activation(out=dst, in_=src, func=AF.Copy)
                else:
                    nc.vector.tensor_copy(dst, src)

            # ------------- MLP -------------------------------------------
            def m_hid(st, mh):
                if mh == 0:
                    st["hid"] = gp3.tile([128, MH, GTOK], FP8, tag="hid",
                                        name=f"hid{st['idx']}")
                hsl = slice(mh * 128, (mh + 1) * 128)
                psh = ps.tile([128, GTOK], F32, tag="ps", name="psh")
                for j in range(2):
                    nc.tensor.matmul(psh, w1_t[:, 2 * j:2 * j + 2, hsl],
                                     st["xn2_cm"][:, 2 * j:2 * j + 2, :],
                                     start=(j == 0), stop=(j == 1),
                                     perf_mode=DR)
                nc.scalar.activation(out=st["hid"][:, mh, :], in_=psh,
                                     func=AF.Gelu_apprx_tanh, scale=IWS)

            def m_out(st, it, eng):
                tsl = slice(it * 128, (it + 1) * 128)
                pso = ps.tile([128, C], F32, tag="ps", name="pso")
                for j in range(MH // 2):
                    nc.tensor.matmul(pso,
                                     st["hid"][:, 2 * j:2 * j + 2, tsl],
                                     w2_t[:, 2 * j:2 * j + 2, :],
                                     start=(j == 0), stop=False,
                                     perf_mode=DR)
                nc.tensor.matmul(pso, ones_t, b2_t, start=False, stop=True)
                nc.vector.scalar_tensor_tensor(
                    out=st["x_tm"][:, it, :], in0=pso, scalar=IWS,
                    in1=st["x2_tm"][:, it, :], op0=OP.mult, op1=OP.add)
                row0 = (st["idx"] % NG) * GTOK + it * 128
                nc.sync.dma_start(out=out_d[row0:row0 + 128, :],
                                  in_=st["x_tm"][:, it, :])

            # ------------- pipeline --------------------------------------
            def head(st):
                for it in range(TPG):
                    a_load(st, it)
                a_stats(st)
                for it in range(TPG):
                    a_apply(st, it)
                a_copy(st, 0, "act")
                a_copy(st, 1, "dve")
                for m in range(KT):
                    b_proj(st, m)

            head(insts[0])
            load_late_weights()

            MO = {"dp": ("dve", "pool", "dve", "pool"),
                  "pp": ("pool", "pool", "pool", "pool"),
                  "dd": ("dve", "dve", "dve", "dve")}[CFG["mout"]]

            for k in range(N + 2):
                g = insts[k] if k < N else None
                gA = insts[k - 1] if 1 <= k <= N else None       # tail part 1
                gB = insts[k - 2] if k >= 2 else None            # tail part 2
                gC = insts[k + 1] if k + 1 < N else None         # head
                if g:
                    for m in range(KT):
                        s_p1(g, m)
                    s_round_route(g)
                if gB:
                    for mh in range(6, 9):
                        m_hid(gB, mh)
                if g:
                    s_round_y(g)      # round 1
                    s_round_mm(g)
                    s_round_route(g)
                if gA:
                    r_pair(gA, 0, "dve")
                if gB:
                    for mh in range(9, MH):
                        m_hid(gB, mh)
                if g:
                    s_round_y(g)      # round 2
                    s_round_mm(g)
                    s_round_route(g)
                if gA:
                    r_pair(gA, 1, "pool")
                    n2_load(gA, 0)
                    n2_load(gA, 1)
                if gB:
                    m_out(gB, 0, MO[0])
                    m_out(gB, 1, MO[1])
                if g:
                    s_round_y(g)      # round 3
                    s_round_mm(g)
                    s_round_route(g)
                if gA:
                    n2_load(gA, 2)
                    n2_load(gA, 3)
                    n2_stats(gA)
                if gB:
                    m_out(gB, 2, MO[2])
                    m_out(gB, 3, MO[3])
                if gC:
                    a_load(gC, 0)
                    a_load(gC, 1)
                if g:
                    s_round_y(g)      # round 4
                    s_round_mm(g)
                    s_round_route(g)
                if gA:
                    for it in range(TPG):
                        n2_apply(gA, it)
                if gC:
                    a_load(gC, 2)
                    a_load(gC, 3)
                    a_stats(gC)
                if g:
                    s_round_y(g)      # round 5
                    s_round_mm(g)
                    s_round_route(g)
                if gA:
                    n2_copy(gA, 0, "act")
                    n2_copy(gA, 1, "dve")
                if gC:
                    for it in range(TPG):
                        a_apply(gC, it)
                if g:
                    s_round_y(g)      # round 6
                    s_round_mm(g)
                    s_final_route(g)
                if gA:
                    for mh in range(6):
                        m_hid(gA, mh)
                if g:
                    s_final(g)
                if gC:
                    a_copy(gC, 0, "act")
                    a_copy(gC, 1, "dve")
                    for m in range(KT):
                        b_proj(gC, m)
    return nc


# ----------------------------------------------------------- device build --
def build_nc(repeat=1):
    nc = bass.Bass()

    x_in = nc.declare_dram_parameter("x", [NTOK, C], F32, isOutput=False)
    wu_d = nc.declare_dram_parameter("wu", [128, 4, C], FP8, isOutput=False)
    wg_d = nc.declare_dram_parameter("wg", [128, 4, C], FP8, isOutput=False)
    a_d = nc.declare_dram_parameter("a", [128, 6, C], FP8, isOutput=False)
    w1_d = nc.declare_dram_parameter("w1", [128, 4, HID], FP8, isOutput=False)
    w2_d = nc.declare_dram_parameter("w2", [128, MH, C], FP8, isOutput=False)
    ones_d = nc.declare_dram_parameter("ones", [1, 128], FP8, isOutput=False)
    b2_d = nc.declare_dram_parameter("b2", [1, C], FP8, isOutput=False)
    pad_d = nc.declare_dram_parameter("pad", [128, GTOK], FP8, isOutput=False)
    eye_d = nc.declare_dram_parameter("eye", [128, 128], BF16, isOutput=False)
    out_d = nc.declare_dram_parameter("out", [NTOK, C], F32, isOutput=True)

    N = NG * repeat

    with TileContext(nc) as tc:
        with (
            tc.tile_pool(name="wp", bufs=1) as wp,
            tc.tile_pool(name="gp", bufs=2) as gp,
            tc.tile_pool(name="gp3", bufs=3) as gp3,
            tc.tile_pool(name="xp", bufs=4) as xp,
            tc.tile_pool(name="tp", bufs=2) as tp,
            tc.tile_pool(name="sp", bufs=2) as sp,
            tc.tile_pool(name="ps", bufs=2, space="PSUM") as ps,
            tc.tile_pool(name="scanp", bufs=1, space="PSUM") as scanp,
            tc.tile_pool(name="pstp", bufs=3, space="PSUM") as pstp,
        ):
            # ---- weights / constants (loaded once) ----
            wu_t = wp.tile([128, 4, C], FP8, tag="wu")
            wg_t = wp.tile([128, 4, C], FP8, tag="wg")
            a_t = wp.tile([128, 6, C], FP8, tag="a")
            w1_t = wp.tile([128, 4, HID], FP8, tag="w1")
            w2_t = wp.tile([128, MH, C], FP8, tag="w2")
            ones_t = wp.tile([1, 128], FP8, tag="ones")
            b2_t = wp.tile([1, C], FP8, tag="b2")
            eye_t = wp.tile([128, 128], BF16, tag="eye")
            nc.sync.dma_start(out=eye_t, in_=eye_d[:, :])
            nc.sync.dma_start(out=wu_t, in_=wu_d[:, :, :])
            nc.sync.dma_start(out=wg_t, in_=wg_d[:, :, :])
            nc.sync.dma_start(out=a_t, in_=a_d[:, :, :])

            def load_late_weights():
                nc.sync.dma_start(out=w1_t, in_=w1_d[:, :, :])
                nc.sync.dma_start(out=w2_t, in_=w2_d[:, :, :])
                nc.sync.dma_start(out=ones_t, in_=ones_d[:, :])
                nc.sync.dma_start(out=b2_t, in_=b2_d[:, :])

            insts = [dict(idx=i) for i in range(N)]

            # ------------- phase A: load + LN1 stats/apply + transpose ----
            def a_load(st, it):
                if it == 0:
                    st["x_tm"] = xp.tile([128, TPG, C], F32, tag="xtm",
                                         name=f"xtm{st['idx']}")
                    st["mvb"] = sp.tile([128, TPG, 2], F32, tag="mvb",
                                        name=f"mvb{st['idx']}")
                row0 = (st["idx"] % NG) * GTOK + it * 128
                nc.sync.dma_start(out=st["x_tm"][:, it, :],
                                  in_=x_in[row0:row0 + 128, :])
                mv6 = sp.tile([128, 6], F32, tag="mv6", bufs=2)
                nc.vector.bn_stats(out=mv6, in_=st["x_tm"][:, it, :])
                nc.vector.bn_aggr(out=st["mvb"][:, it, :], in_=mv6)

            def _rsqrt(mvb, tag):
                """r1 = rsqrt(var+eps), mnr = -mean*r1, each [128,TPG,1]."""
                S = [128, TPG, 1]
                vpe = sp.tile(S, F32, tag=f"vpe{tag}")
                nc.vector.tensor_scalar(out=vpe, in0=mvb[:, :, 1:2],
                                        scalar1=EPS, scalar2=None, op0=OP.add)
                i1 = sp.tile(S, I32, tag=f"i1{tag}")
                nc.vector.tensor_scalar(out=i1, in0=vpe.bitcast(I32),
                                        scalar1=1, scalar2=None,
                                        op0=OP.logical_shift_right)
                i2 = sp.tile(S, I32, tag=f"i2{tag}")
                nc.vector.tensor_scalar(out=i2, in0=i1, scalar1=MAGIC,
                                        scalar2=-1, op0=OP.subtract,
                                        op1=OP.mult)
                r0 = i2.bitcast(F32)
                t = sp.tile(S, F32, tag=f"t{tag}")
                nc.vector.tensor_mul(out=t, in0=vpe, in1=r0)
                nc.vector.tensor_mul(out=t, in0=t, in1=r0)
                nc.vector.tensor_scalar(out=t, in0=t, scalar1=-0.5,
                                        scalar2=1.5, op0=OP.mult, op1=OP.add)
                rA = sp.tile(S, F32, tag=f"rA{tag}")
                nc.vector.tensor_mul(out=rA, in0=t, in1=r0)
                t2 = sp.tile(S, F32, tag=f"t2{tag}")
                nc.vector.tensor_mul(out=t2, in0=vpe, in1=rA)
                nc.vector.tensor_mul(out=t2, in0=t2, in1=rA)
                nc.vector.tensor_scalar(out=t2, in0=t2, scalar1=-0.5,
                                        scalar2=1.5, op0=OP.mult, op1=OP.add)
                r1 = sp.tile(S, F32, tag=f"r1{tag}")
                nc.vector.tensor_mul(out=r1, in0=t2, in1=rA)
                nr = sp.tile(S, F32, tag=f"nr{tag}")
                nc.vector.tensor_scalar(out=nr, in0=r1, scalar1=-1.0,
                                        scalar2=None, op0=OP.mult)
                mnr = sp.tile(S, F32, tag=f"mnr{tag}")
                nc.vector.tensor_mul(out=mnr, in0=mvb[:, :, 0:1], in1=nr)
                return r1, mnr

            def a_stats(st):
                st["r1"], st["mnr"] = _rsqrt(st["mvb"], "a")

            def a_apply(st, it):
                xn = tp.tile([128, C], BF16, tag="xntm", bufs=2)
                if CFG["apply1"] == "act":
                    nc.scalar.activation(out=xn, in_=st["x_tm"][:, it, :],
                                         func=AF.Identity,
                                         scale=st["r1"][:, it, :],
                                         bias=st["mnr"][:, it, :])
                else:
                    nc.gpsimd.tensor_scalar(out=xn, in0=st["x_tm"][:, it, :],
                                            scalar1=st["mvb"][:, it, 0:1],
                                            scalar2=st["r1"][:, it, :],
                                            op0=OP.subtract, op1=OP.mult)
                pair, half = divmod(it, 2)
                if half == 0:
                    st[f"pst_a{pair}"] = pstp.tile([128, KT, 2, 128], BF16,
                                                   tag="pst", name="psta")
                pt = st[f"pst_a{pair}"]
                for c in range(KT):
                    nc.tensor.transpose(pt[:, c, half, :],
                                        xn[:, c * 128:(c + 1) * 128], eye_t)

            def a_copy(st, pair, eng):
                if pair == 0:
                    st["xn_cm"] = gp.tile([128, 4, GTOK], FP8, tag="xncm",
                                          name=f"xncm{st['idx']}")
                    nc.sync.dma_start(out=st["xn_cm"][:, 3, :],
                                      in_=pad_d[:, :])
                src = st[f"pst_a{pair}"]
                dst = st["xn_cm"][:, 0:KT, pair * 256:(pair + 1) * 256]
                dst = dst.rearrange("p c (i q) -> p c i q", i=2)
                if eng == "act":
                    nc.scalar.activation(out=dst, in_=src, func=AF.Copy)
                else:
                    nc.vector.tensor_copy(dst, src)

            # ------------- phase B: u/g projections, gate, m2 -------------
            def b_proj(st, m):
                if m == 0:
                    st["ybuf"] = gp.tile([128, 6, GTOK], FP8, tag="ybuf",
                                         name=f"ybuf{st['idx']}")
                    THDT = F32 if CFG["thp_f32"] else BF16
                    st["thp"] = gp.tile([128, KT, GTOK], THDT, tag="thp",
                                        name=f"thp{st['idx']}")
                    st["thm"] = gp.tile([128, KT, GTOK], THDT, tag="thm",
                                        name=f"thm{st['idx']}")
                    st["m2b"] = gp.tile([128, KT, GTOK], BF16, tag="m2b",
                                        name=f"m2b{st['idx']}")
                msl = slice(m * 128, (m + 1) * 128)
                xn_cm = st["xn_cm"]
                psu = ps.tile([128, GTOK], F32, tag="ps", name="psu")
                for j in range(2):
                    nc.tensor.matmul(psu, wu_t[:, 2 * j:2 * j + 2, msl],
                                     xn_cm[:, 2 * j:2 * j + 2, :],
                                     start=(j == 0), stop=(j == 1),
                                     perf_mode=DR)
                psg = ps.tile([128, GTOK], F32, tag="ps", name="psg")
                for j in range(2):
                    nc.tensor.matmul(psg, wg_t[:, 2 * j:2 * j + 2, msl],
                                     xn_cm[:, 2 * j:2 * j + 2, :],
                                     start=(j == 0), stop=(j == 1),
                                     perf_mode=DR)
                thraw = tp.tile([128, GTOK],
                                F32 if CFG["thp_f32"] else BF16,
                                tag="thraw", bufs=2)
                nc.scalar.activation(out=thraw, in_=psg, func=AF.Tanh,
                                     scale=SH)
                te = nc.vector if CFG["thp_eng"] == "dve" else nc.gpsimd
                te.tensor_scalar(out=st["thp"][:, m, :], in0=thraw,
                                 scalar1=1.0, scalar2=None, op0=OP.add)
                tm = nc.vector if CFG["pool_off"] else nc.gpsimd
                tm.tensor_scalar(out=st["thm"][:, m, :], in0=thraw,
                                 scalar1=-1.0, scalar2=None, op0=OP.add)
                nc.vector.scalar_tensor_tensor(
                    out=st["m2b"][:, m, :], in0=psu, scalar=SH,
                    in1=st["thm"][:, m, :], op0=OP.mult, op1=OP.mult)
                nc.scalar.activation(out=st["ybuf"][:, KT + m, :],
                                     in_=st["m2b"][:, m, :], func=AF.Copy)

            # ------------- scan ------------------------------------------
            def s_p1(st, m):
                if m == 0:
                    st["p"] = scanp.tile([128, KT, GTOK], F32, tag="p",
                                         name="scp")
                msl = slice(m * 128, (m + 1) * 128)
                y = st["ybuf"]
                nc.tensor.matmul(st["p"][:, m, :], a_t[:, 3:5, msl],
                                 y[:, 3:5, :], start=True, stop=False,
                                 perf_mode=DR)
                nc.tensor.matmul(st["p"][:, m, :], a_t[:, 5, msl],
                                 y[:, 5, :], start=False, stop=True)

            def s_round_route(st):
                """ACT+Pool route for bank 2 of the NEXT y — emitted right
                after the matmuls producing p so the ACT queue reaches it
                before any fill."""
                return

            def s_round_y(st):
                p, y, thp = st["p"], st["ybuf"], st["thp"]
                nc.vector.scalar_tensor_tensor(
                    out=y[:, 0:KT, :], in0=p, scalar=SH, in1=thp,
                    op0=OP.mult, op1=OP.mult)

            def s_round_mm(st):
                p, y = st["p"], st["ybuf"]
                for m in range(KT):
                    msl = slice(m * 128, (m + 1) * 128)
                    for j in range(KT):
                        nc.tensor.matmul(p[:, m, :],
                                         a_t[:, 2 * j:2 * j + 2, msl],
                                         y[:, 2 * j:2 * j + 2, :],
                                         start=(j == 0), stop=(j == KT - 1),
                                         perf_mode=DR)

            def s_final_route(st):
                t1 = tp.tile([128, KT, GTOK], BF16, tag="t1", bufs=2,
                             name="t1f")
                st["t1"] = t1

            def s_final(st):
                p, y, thp, t1 = st["p"], st["ybuf"], st["thp"], st["t1"]
                nc.vector.scalar_tensor_tensor(
                    out=t1, in0=p, scalar=SH, in1=thp,
                    op0=OP.mult, op1=OP.mult)
                h = gp.tile([128, KT, GTOK], BF16, tag="hcm",
                            name=f"hcm{st['idx']}")
                st["h_cm"] = h
                m2b = st["m2b"]
                if CFG["pool_off"]:
                    nc.vector.tensor_add(out=h, in0=t1, in1=m2b)
                else:
                    nc.vector.tensor_add(out=h[:, 0:1, :], in0=t1[:, 0:1, :],
                                         in1=m2b[:, 0:1, :])
                    nc.gpsimd.tensor_add(out=h[:, 1:3, :],
                                         in0=t1[:, 1:3, :],
                                         in1=m2b[:, 1:3, :])

            # ------------- residual 1 ------------------------------------
            def r_pair(st, pair, eng):
                if pair == 0:
                    st["x2_tm"] = gp3.tile([128, TPG, C], F32, tag="x2tm",
                                          name=f"x2tm{st['idx']}")
                pst = pstp.tile([128, KT, 2, 128], BF16, tag="pst",
                                name="pstr")
                for half in range(2):
                    it = pair * 2 + half
                    tsl = slice(it * 128, (it + 1) * 128)
                    for c in range(KT):
                        nc.tensor.transpose(pst[:, c, half, :],
                                            st["h_cm"][:, c, tsl], eye_t)
                dst = st["x2_tm"][:, pair * 2:pair * 2 + 2, :]
                dst = dst.rearrange("p i (c q) -> p i c q", c=KT)
                src = st["x_tm"][:, pair * 2:pair * 2 + 2, :]
                src = src.rearrange("p i (c q) -> p i c q", c=KT)
                hsrc = pst.rearrange("p c i q -> p i c q")
                if eng == "dve" or CFG["pool_off"]:
                    nc.vector.tensor_sub(out=dst, in0=src, in1=hsrc)
                else:
                    hb = tp.tile([128, 2, KT, 128], BF16, tag="hb", bufs=2)
                    nc.scalar.activation(out=hb, in_=hsrc, func=AF.Copy)
                    nc.gpsimd.tensor_sub(out=dst, in0=src, in1=hb)

            # ------------- norm 2 ----------------------------------------
            def n2_load(st, it):
                if it == 0:
                    st["mvb2"] = sp.tile([128, TPG, 2], F32, tag="mvb2",
                                         name=f"mvb2{st['idx']}")
                mv6 = sp.tile([128, 6], F32, tag="mv6", bufs=2)
                nc.vector.bn_stats(out=mv6, in_=st["x2_tm"][:, it, :])
                nc.vector.bn_aggr(out=st["mvb2"][:, it, :], in_=mv6)

            def n2_stats(st):
                st["r2"], st["mnr2"] = _rsqrt(st["mvb2"], "b")

            def n2_apply(st, it):
                xn2 = tp.tile([128, C], BF16, tag="xn2tm", bufs=2)
                if CFG["pool_off"]:
                    nc.scalar.activation(out=xn2, in_=st["x2_tm"][:, it, :],
                                         func=AF.Identity,
                                         scale=st["r2"][:, it, :],
                                         bias=st["mnr2"][:, it, :])
                else:
                    nc.gpsimd.tensor_scalar(out=xn2,
                                            in0=st["x2_tm"][:, it, :],
                                            scalar1=st["mvb2"][:, it, 0:1],
                                            scalar2=st["r2"][:, it, :],
                                            op0=OP.subtract, op1=OP.mult)
                pair, half = divmod(it, 2)
                if half == 0:
                    st[f"pst_n{pair}"] = pstp.tile([128, KT, 2, 128], BF16,
                                                   tag="pst", name="pstn")
                pt = st[f"pst_n{pair}"]
                for c in range(KT):
                    nc.tensor.transpose(pt[:, c, half, :],
                                        xn2[:, c * 128:(c + 1) * 128], eye_t)

            def n2_copy(st, pair, eng):
                if pair == 0:
                    st["xn2_cm"] = gp3.tile([128, 4, GTOK], FP8, tag="xn2cm",
                                           name=f"xn2cm{st['idx']}")
                    nc.sync.dma_start(out=st["xn2_cm"][:, 3, :],
                                      in_=pad_d[:, :])
                src = st[f"pst_n{pair}"]
                dst = st["xn2_cm"][:, 0:KT, pair * 256:(pair + 1) * 256]
                dst = dst.rearrange("p c (i q) -> p c i q", i=2)
                if eng == "act":
                    nc.scalar.activation(out=dst, in_=src, func=AF.Copy)
                else:
                    nc.vector.tensor_copy(dst, src)

            # ------------- MLP -------------------------------------------
            def m_hid(st, mh):
                if mh == 0:
                    st["hid"] = gp3.tile([128, MH, GTOK], FP8, tag="hid",
                                        name=f"hid{st['idx']}")
                hsl = slice(mh * 128, (mh + 1) * 128)
                psh = ps.tile([128, GTOK], F32, tag="ps", name="psh")
                for j in range(2):
                    nc.tensor.matmul(psh, w1_t[:, 2 * j:2 * j + 2, hsl],
                                     st["xn2_cm"][:, 2 * j:2 * j + 2, :],
                                     start=(j == 0), stop=(j == 1),
                                     perf_mode=DR)
                nc.scalar.activation(out=st["hid"][:, mh, :], in_=psh,
                                     func=AF.Gelu_apprx_tanh, scale=IWS)

            def m_out(st, it, eng):
                tsl = slice(it * 128, (it + 1) * 128)
                pso = ps.tile([128, C], F32, tag="ps", name="pso")
                for j in range(MH // 2):
                    nc.tensor.matmul(pso,
                                     st["hid"][:, 2 * j:2 * j + 2, tsl],
                                     w2_t[:, 2 * j:2 * j + 2, :],
                                     start=(j == 0), stop=False,
                                     perf_mode=DR)
                nc.tensor.matmul(pso, ones_t, b2_t, start=False, stop=True)
                nc.vector.scalar_tensor_tensor(
                    out=st["x_tm"][:, it, :], in0=pso, scalar=IWS,
                    in1=st["x2_tm"][:, it, :], op0=OP.mult, op1=OP.add)
                row0 = (st["idx"] % NG) * GTOK + it * 128
                nc.sync.dma_start(out=out_d[row0:row0 + 128, :],
                                  in_=st["x_tm"][:, it, :])

            # ------------- pipeline --------------------------------------
            def head(st):
                for it in range(TPG):
                    a_load(st, it)
                a_stats(st)
                for it in range(TPG):
                    a_apply(st, it)
                a_copy(st, 0, "act")
                a_copy(st, 1, "dve")
                for m in range(KT):
                    b_proj(st, m)

            head(insts[0])
            load_late_weights()

            for k in range(N + 1):
                g = insts[k] if k < N else None
                gprev = insts[k - 1] if k >= 1 else None
                gnext = insts[k + 1] if k + 1 < N else None
                if g:
                    for m in range(KT):
                        s_p1(g, m)
                    s_round_route(g)
                if gprev:
                    r_pair(gprev, 0, "dve")
                if g:
                    s_round_y(g)      # round 1
                    s_round_mm(g)
                    s_round_route(g)
                if gprev:
                    r_pair(gprev, 1, "pool")
                    n2_load(gprev, 0)
                    n2_load(gprev, 1)
                if g:
                    s_round_y(g)      # round 2
                    s_round_mm(g)
                    s_round_route(g)
                if gprev:
                    n2_load(gprev, 2)
                    n2_load(gprev, 3)
                    n2_stats(gprev)
                    n2_apply(gprev, 0)
                    n2_apply(gprev, 1)
                if g:
                    s_round_y(g)      # round 3
                    s_round_mm(g)
                    s_round_route(g)
                if gprev:
                    n2_apply(gprev, 2)
                    n2_apply(gprev, 3)
                    n2_copy(gprev, 0, "act")
                    n2_copy(gprev, 1, "dve")
                    for mh in range(3):
                        m_hid(gprev, mh)
                if g:
                    s_round_y(g)      # round 4
                    s_round_mm(g)
                    s_round_route(g)
                if gprev:
                    for mh in range(3, 7):
                        m_hid(gprev, mh)
                if gnext:
                    a_load(gnext, 0)
                    a_load(gnext, 1)
                if g:
                    s_round_y(g)      # round 5
                    s_round_mm(g)
                    s_round_route(g)
                if gprev:
                    for mh in range(7, MH):
                        m_hid(gprev, mh)
                if gnext:
                    a_load(gnext, 2)
                    a_load(gnext, 3)
                    a_stats(gnext)
                if g:
                    s_round_y(g)      # round 6
                    s_round_mm(g)
                    s_final_route(g)
                if gprev:
                    mo = {"dp": ("dve", "pool", "dve", "pool"),
                          "pp": ("pool", "pool", "pool", "pool"),
                          "dd": ("dve", "dve", "dve", "dve")}[CFG["mout"]]
                    for _it in range(TPG):
                        m_out(gprev, _it, mo[_it])
                if g:
                    s_final(g)
                if gnext:
                    for it in range(TPG):
                        a_apply(gnext, it)
                    a_copy(gnext, 0, "act")
                    a_copy(gnext, 1, "dve")
                    for m in range(KT):
                        b_proj(gnext, m)
    return nc


_NC_CACHE = {}


def _get_nc():
    if "nc" not in _NC_CACHE:
        _NC_CACHE["nc"] = _patch_nc(build_nc())
    return _NC_CACHE["nc"]


# ---------------------------------------------------------------- kernel --
def kernel(x, norm1_scale, norm1_bias, Wu, bu, Wg, bg, A,
           norm2_scale, norm2_bias, mlp_w1, mlp_b1, mlp_w2, mlp_b2,
           _return_raw=False):
    import ml_dtypes
    E4 = ml_dtypes.float8_e4m3
    BF = ml_dtypes.bfloat16
    f = np.float32
    x = np.asarray(x, f)
    norm1_scale = np.asarray(norm1_scale, f)
    norm1_bias = np.asarray(norm1_bias, f)
    Wu, bu = np.asarray(Wu, f), np.asarray(bu, f)
    Wg, bg = np.asarray(Wg, f), np.asarray(bg, f)
    A = np.asarray(A, f)
    norm2_scale = np.asarray(norm2_scale, f)
    norm2_bias = np.asarray(norm2_bias, f)
    mlp_w1, mlp_b1 = np.asarray(mlp_w1, f), np.asarray(mlp_b1, f)
    mlp_w2, mlp_b2 = np.asarray(mlp_w2, f), np.asarray(mlp_b2, f)

    # fold LN affine into downstream weights
    wu = norm1_scale[:, None] * Wu
    bu_f = bu + norm1_bias @ Wu
    wg = norm1_scale[:, None] * Wg
    bg_f = bg + norm1_bias @ Wg
    w1 = norm2_scale[:, None] * mlp_w1
    b1_f = mlp_b1 + norm2_bias @ mlp_w1

    def pack3(w, bias):
        """[C,Cout] weights + bias -> [128, 4, Cout] fp8 with bias row."""
        cout = w.shape[1]
        out = np.zeros((128, 4, cout), f)
        for s in range(3):
            out[:, s, :] = w[s * 128:(s + 1) * 128, :]
        out[0, 3, :] = bias
        return (out * WS).astype(E4)

    wu_p = pack3(wu, bu_f)
    wg_p = pack3(wg, bg_f)
    w1_p = pack3(w1, b1_f)

    a_p = np.zeros((128, 6, C), f)
    for s in range(6):
        a_p[:, s, :] = A[(s % 3) * 128:(s % 3) * 128 + 128, :]
    a_p = (a_p * WS).astype(E4)

    w2_p = np.zeros((128, MH, C), f)
    for s in range(MH):
        w2_p[:, s, :] = mlp_w2[s * 128:(s + 1) * 128, :]
    w2_p = (w2_p * WS).astype(E4)

    b2_p = (mlp_b2[None, :] * WS).astype(E4)
    ones_p = np.ones((1, 128), f).astype(E4)
    pad_p = np.zeros((128, GTOK), f)
    pad_p[0, :] = 1.0
    pad_p = pad_p.astype(E4)
    eye_p = np.eye(128, dtype=f).astype(BF)

    xs = x.reshape(NCORES, NTOK, C)
    in_maps = [{
        "x": np.ascontiguousarray(xs[i]),
        "wu": wu_p, "wg": wg_p, "a": a_p, "w1": w1_p, "w2": w2_p,
        "ones": ones_p, "b2": b2_p, "pad": pad_p, "eye": eye_p,
    } for i in range(NCORES)]

    res = run_bass_kernel_spmd(_get_nc(), in_maps, list(range(NCORES)))
    if _return_raw:
        return res
    out = np.concatenate([res.results[i]["out"] for i in range(NCORES)],
                         axis=0)
    return out.reshape(B, H, W, C).astype(np.float32)



# revision 26
# speedup vs baseline: 1.1335x; 1.1335x over previous
"""CSSM TinyViT block on 8 TRN2 NeuronCores — DMA-xbar-transpose version.

Strategy
--------
Data-parallel over batch: B=16 -> 2 samples (2048 tokens) per core, processed
as 4 groups of 512 tokens in a software pipeline.

All channel-mixing matmuls run fp8(e4m3) DoubleRow.  Token-major -> channel-
major layout changes go through the DMA crossbar transpose (u16 views of fp8
pairs; the channel order becomes (256c + 2p + s), which is folded into the
weight row order host-side).  h comes back token-major the same way in bf16.
This removes every PE transpose and every PSUM->SBUF copy of the baseline.

Biases ride the epilogues: bg via the tanh ACT bias AP, bu via the m2 STT
scalar AP, b1 via the gelu ACT bias AP, b2 via the ones-row matmul.

Engine placement: DVE keeps only PSUM-coupled elementwise work (scan gate
multiplies, m2, m_out epilogue, bn_stats); the LN rsqrt chains (rsqrt bit
hack + 1 Newton step), the residual subtract and the xn pad memsets run on
Pool; LN applies, tanh/gelu and the m2->fp8 copies run on ACT.

The emission order interleaves PE-decoupled DVE filler (proj epilogues,
bn_stats of neighbour groups, m_out) between the scan rounds so the DVE never
waits on the scan matmuls.

PSUM: scan p [128,3,512] f32 (3 banks) + 4-deep 1-bank ring (proj/MLP) = 7.
"""
import json
import os
import types

import numpy as np

import concourse.bass as bass
import concourse.mybir as mybir
from concourse.tile import TileContext
from concourse.bass_utils import run_bass_kernel_spmd

F32 = mybir.dt.float32
FP8 = mybir.dt.float8e4
BF16 = mybir.dt.bfloat16
U16 = mybir.dt.uint16
I32 = mybir.dt.int32
AF = mybir.ActivationFunctionType
OP = mybir.AluOpType
DR = mybir.MatmulPerfMode.DoubleRow

B, H, W, C, T = 16, 32, 32, 384, 8
HID = 4 * C
EPS = 1e-6
NCORES = 8
BSH = B // NCORES
NTOK = BSH * H * W             # 2048 tokens per core
GTOK = 512                     # tokens per group
NG = NTOK // GTOK              # 4 groups
TPG = GTOK // 128              # 4 token-tiles per group
KT = C // 128                  # 3 channel tiles
MH = HID // 128                # 12 hidden tiles
WS = 64.0                      # fp8 weight scale
SH = 0.5 / WS                  # dequant incl. the tanh 0.5
IWS = 1.0 / WS
MAGIC = 0x5F3759DF
CPAD = 512                     # xn padded channel count (u16-transposable)


# ---------------------------------------------------------------- bir fix --
# This container's walrus rejects instructions whose sync-wait list exceeds
# the opcode's wait slots.  Move excess waits onto EventSemaphore
# instructions inserted before the instruction on the same engine queue.
_WAIT_LIMITS = {"Drain": 0, "DmaTransposeAnt": 0}
_WAIT_DEFAULT = 1


def _fix_bir_json(bj: bytes) -> bytes:
    bir = json.loads(bj)
    counter = [0]

    def fix_blocks(blocks):
        for b in blocks:
            insts = b.get("instructions")
            if insts:
                new = []
                for inst in insts:
                    si = inst.get("sync_info")
                    waits = (si or {}).get("on_wait") or []
                    limit = _WAIT_LIMITS.get(inst.get("opcode"), _WAIT_DEFAULT)
                    if len(waits) > limit:
                        n_extra = len(waits) - limit
                        extra, keep = waits[:n_extra], waits[n_extra:]
                        for wv in extra:
                            counter[0] += 1
                            new.append({
                                "name": f"I-wfix-{counter[0]}",
                                "opcode": "EventSemaphore",
                                "engine": inst["engine"],
                                "ins": [],
                                "outs": [],
                                "sync_info": {"on_update": [], "on_wait": [wv]},
                                "debug": inst.get("debug", 0),
                            })
                        si["on_wait"] = keep
                    new.append(inst)
                b["instructions"] = new
            fix_blocks(b.get("blocks") or [])

    for fn in bir.get("functions", []):
        fix_blocks(fn.get("blocks") or [])
    return json.dumps(bir).encode()


def _patch_nc(nc):
    orig = nc.to_json_bytes

    def to_json_bytes(self):
        return _fix_bir_json(orig())

    nc.to_json_bytes = types.MethodType(to_json_bytes, nc)
    return nc


# ----------------------------------------------------------- device build --
def build_nc(repeat=1):
    nc = bass.Bass()

    x_in = nc.declare_dram_parameter("x", [NTOK, C], F32, isOutput=False)
    wu_d = nc.declare_dram_parameter("wu", [128, 4, C], FP8, isOutput=False)
    wg_d = nc.declare_dram_parameter("wg", [128, 4, C], FP8, isOutput=False)
    a_d = nc.declare_dram_parameter("a", [128, 6, C], FP8, isOutput=False)
    w1_d = nc.declare_dram_parameter("w1", [128, 4, HID], FP8, isOutput=False)
    w2_d = nc.declare_dram_parameter("w2", [128, MH, C], FP8, isOutput=False)
    ones_d = nc.declare_dram_parameter("ones", [1, 128], FP8, isOutput=False)
    b2_d = nc.declare_dram_parameter("b2", [1, C], FP8, isOutput=False)
    bu_d = nc.declare_dram_parameter("bu", [128, KT], F32, isOutput=False)
    bg_d = nc.declare_dram_parameter("bg", [128, KT], F32, isOutput=False)
    b1_d = nc.declare_dram_parameter("b1", [128, MH], F32, isOutput=False)
    cst_d = nc.declare_dram_parameter("cst", [128, 6, TPG, 1], F32,
                                      isOutput=False)
    out_d = nc.declare_dram_parameter("out", [NTOK, C], F32, isOutput=True)

    N = NG * repeat

    with TileContext(nc) as tc:
        with (
            tc.tile_pool(name="wp", bufs=1) as wp,
            tc.tile_pool(name="xp", bufs=4) as xp,
            tc.tile_pool(name="x2p", bufs=2) as x2p,
            tc.tile_pool(name="gp", bufs=2) as gp,
            tc.tile_pool(name="tp", bufs=2) as tp,
            tc.tile_pool(name="sp", bufs=2) as sp,
            tc.tile_pool(name="ps", bufs=3, space="PSUM") as ps,
            tc.tile_pool(name="scanp", bufs=2, space="PSUM") as scanp,
        ):
            # ---- weights / constants ----
            wu_t = wp.tile([128, 4, C], FP8, tag="wu")
            wg_t = wp.tile([128, 4, C], FP8, tag="wg")
            a_t = wp.tile([128, 6, C], FP8, tag="a")
            w1_t = wp.tile([128, 4, HID], FP8, tag="w1")
            w2_t = wp.tile([128, MH, C], FP8, tag="w2")
            ones_t = wp.tile([1, 128], FP8, tag="ones")
            b2_t = wp.tile([1, C], FP8, tag="b2")
            bu_t = wp.tile([128, KT], F32, tag="bu")
            bg_t = wp.tile([128, KT], F32, tag="bg")
            b1_t = wp.tile([128, MH], F32, tag="b1")
            cst_t = wp.tile([128, 6, TPG, 1], F32, tag="cst")
            nc.sync.dma_start(out=cst_t, in_=cst_d[:, :, :, :])
            nc.sync.dma_start(out=wu_t, in_=wu_d[:, :, :])
            nc.sync.dma_start(out=wg_t, in_=wg_d[:, :, :])
            nc.sync.dma_start(out=a_t, in_=a_d[:, :, :])
            nc.sync.dma_start(out=bu_t, in_=bu_d[:, :])
            nc.sync.dma_start(out=bg_t, in_=bg_d[:, :])
            nc.sync.dma_start(out=b1_t, in_=b1_d[:, :])

            def load_late_weights():
                nc.sync.dma_start(out=w1_t, in_=w1_d[:, :, :])
                nc.sync.dma_start(out=w2_t, in_=w2_d[:, :, :])
                nc.sync.dma_start(out=ones_t, in_=ones_d[:, :])
                nc.sync.dma_start(out=b2_t, in_=b2_d[:, :])

            insts = [dict(idx=i) for i in range(N)]

            # ---------------- load + LN1 stats -----------------------------
            def ld(st):
                st["x_tm"] = xp.tile([128, TPG, C], F32, tag="xtm",
                                     name=f"xtm{st['idx']}")
                row0 = (st["idx"] % NG) * GTOK
                for it in range(TPG):
                    r = row0 + it * 128
                    nc.sync.dma_start(out=st["x_tm"][:, it, :],
                                      in_=x_in[r:r + 128, :])

            def stats(st, which):
                src = st["x_tm"] if which == 1 else st["x2_tm"]
                mvb = sp.tile([128, TPG, 2], F32, tag=f"mvb{which}",
                              name=f"mvb{which}{st['idx']}")
                for it in range(TPG):
                    st6 = sp.tile([128, 6], F32, tag=f"st6{which}", bufs=2)
                    nc.vector.bn_stats(out=st6, in_=src[:, it, :])
                    nc.vector.bn_aggr(out=mvb[:, it, :], in_=st6)
                st[f"mvb{which}"] = mvb

            def rsq(st, which):
                """Pool: r1 = rsqrt(var+eps), mnr = -mean*r1 (2 Newton).

                Pool only passes the ISA check for TensorTensor/TensorCopy/
                Memset, so every scalar rides a broadcast constant tile
                (cst_t[:, j] = [128, TPG, 1]): 0=eps 1=int(1) 2=magic
                3=-0.5 4=1.5 5=-1.
                """
                mvb = st[f"mvb{which}"]
                S = [128, TPG, 1]
                tg = f"rq{which}"
                TT = nc.gpsimd.tensor_tensor
                # eps-add + bit hack on DVE (Pool shifts need i64 out);
                # the Newton steps run on Pool.
                vpe = sp.tile(S, F32, tag=f"vpe{tg}")
                nc.vector.tensor_scalar(out=vpe, in0=mvb[:, :, 1:2],
                                        scalar1=EPS, scalar2=None, op0=OP.add)
                i1 = sp.tile(S, I32, tag=f"i1{tg}")
                nc.vector.tensor_scalar(out=i1, in0=vpe.bitcast(I32),
                                        scalar1=1, scalar2=None,
                                        op0=OP.logical_shift_right)
                i2 = sp.tile(S, I32, tag=f"i2{tg}")
                nc.vector.tensor_scalar(out=i2, in0=i1, scalar1=MAGIC,
                                        scalar2=-1, op0=OP.subtract,
                                        op1=OP.mult)
                r0 = i2.bitcast(F32)
                t = sp.tile(S, F32, tag=f"t{tg}")
                TT(out=t, in0=vpe, in1=r0, op=OP.mult)
                TT(out=t, in0=t, in1=r0, op=OP.mult)
                TT(out=t, in0=t, in1=cst_t[:, 3], op=OP.mult)
                TT(out=t, in0=t, in1=cst_t[:, 4], op=OP.add)
                r1 = sp.tile(S, F32, tag=f"r1{tg}",
                             name=f"r1{tg}{st['idx']}")
                TT(out=r1, in0=t, in1=r0, op=OP.mult)
                t2 = sp.tile(S, F32, tag=f"t2{tg}")
                TT(out=t2, in0=vpe, in1=r1, op=OP.mult)
                TT(out=t2, in0=t2, in1=r1, op=OP.mult)
                TT(out=t2, in0=t2, in1=cst_t[:, 3], op=OP.mult)
                TT(out=t2, in0=t2, in1=cst_t[:, 4], op=OP.add)
                rb = sp.tile(S, F32, tag=f"rb{tg}",
                             name=f"rb{tg}{st['idx']}")
                TT(out=rb, in0=t2, in1=r1, op=OP.mult)
                mm = sp.tile(S, F32, tag=f"mm{tg}")
                TT(out=mm, in0=mvb[:, :, 0:1], in1=rb, op=OP.mult)
                mnr = sp.tile(S, F32, tag=f"mnr{tg}",
                              name=f"mnr{tg}{st['idx']}")
                TT(out=mnr, in0=mm, in1=cst_t[:, 5], op=OP.mult)
                st[f"r{which}"] = rb
                st[f"mnr{which}"] = mnr

            # ---------------- LN apply + DMA transpose ---------------------
            def xn_alloc(st, which):
                xn = gp.tile([128, TPG, CPAD], FP8, tag=f"xn{which}",
                             name=f"xn{which}_{st['idx']}")
                nc.gpsimd.memset(xn[:, :, C:CPAD], 0.0)
                st[f"xn{which}"] = xn

            def apply_ln(st, which):
                src = st["x_tm"] if which == 1 else st["x2_tm"]
                xn = st[f"xn{which}"]
                r1, mnr = st[f"r{which}"], st[f"mnr{which}"]
                for it in range(TPG):
                    nc.scalar.activation(out=xn[:, it, 0:C],
                                         in_=src[:, it, :],
                                         func=AF.Identity,
                                         scale=r1[:, it, :],
                                         bias=mnr[:, it, :])

            def dmat_ln(st, which):
                xn16 = st[f"xn{which}"].bitcast(U16)
                xcm = gp.tile([128, 2, GTOK], U16, tag=f"xcm{which}",
                              name=f"xcm{which}_{st['idx']}")
                for it in range(TPG):
                    nc.scalar.dma_start_transpose(
                        xcm[:, :, it * 128:(it + 1) * 128], xn16[:, it, :])
                st[f"xcm{which}"] = xcm

            def _rhs(st, which):
                """[128, c, 2, 512] fp8 view: (sub s, token t) per chunk."""
                x8 = st[f"xcm{which}"].bitcast(FP8)
                return x8.rearrange("p c (t s) -> p c s t", s=2)

            # ---------------- projections ---------------------------------
            def pjA(st, m):
                """PE psu/psg matmuls + ACT tanh for m-tile."""
                if m == 0:
                    st["ybuf"] = gp.tile([128, 6, GTOK], FP8, tag="ybuf",
                                         name=f"ybuf{st['idx']}")
                    st["thp"] = gp.tile([128, KT, GTOK], BF16, tag="thp",
                                        name=f"thp{st['idx']}")
                    st["m2b"] = gp.tile([128, KT, GTOK], BF16, tag="m2b",
                                        name=f"m2b{st['idx']}")
                msl = slice(m * 128, (m + 1) * 128)
                rhs = _rhs(st, 1)
                psu = ps.tile([128, GTOK], F32, tag="ps", name="psu")
                psg = ps.tile([128, GTOK], F32, tag="ps", name="psg")
                for j in range(2):
                    nc.tensor.matmul(psu, wu_t[:, 2 * j:2 * j + 2, msl],
                                     rhs[:, j, :, :], start=(j == 0),
                                     stop=(j == 1), perf_mode=DR)
                for j in range(2):
                    nc.tensor.matmul(psg, wg_t[:, 2 * j:2 * j + 2, msl],
                                     rhs[:, j, :, :], start=(j == 0),
                                     stop=(j == 1), perf_mode=DR)
                thraw = tp.tile([128, GTOK], BF16, tag="thraw", bufs=2,
                                name=f"thraw{st['idx']}_{m}")
                nc.scalar.activation(out=thraw, in_=psg, func=AF.Tanh,
                                     scale=SH, bias=bg_t[:, m:m + 1])
                st[f"thraw{m}"] = thraw
                st[f"psu{m}"] = psu

            def pjB(st, m):
                """DVE thp/thm/m2 + ACT ybuf copy for m-tile."""
                thraw, psu = st[f"thraw{m}"], st[f"psu{m}"]
                nc.vector.tensor_scalar(out=st["thp"][:, m, :], in0=thraw,
                                        scalar1=1.0, scalar2=SH,
                                        op0=OP.add, op1=OP.mult)
                thm = tp.tile([128, GTOK], BF16, tag="thm", bufs=2,
                              name=f"thm{st['idx']}_{m}")
                nc.vector.tensor_scalar(out=thm, in0=thraw,
                                        scalar1=-1.0, scalar2=SH,
                                        op0=OP.add, op1=OP.mult)
                nc.vector.scalar_tensor_tensor(
                    out=st["m2b"][:, m, :], in0=psu, scalar=bu_t[:, m:m + 1],
                    in1=thm, op0=OP.add, op1=OP.mult)
                nc.scalar.activation(out=st["ybuf"][:, KT + m, :],
                                     in_=st["m2b"][:, m, :], func=AF.Copy)

            # ---------------- scan (two independent half-token chains) ----
            HTOK = GTOK // 2

            def _hsl(hf):
                return slice(hf * HTOK, (hf + 1) * HTOK)

            def sp1(st, hf):
                p = scanp.tile([128, KT, HTOK], F32, tag="p",
                               name=f"scp{hf}")
                st[f"p{hf}"] = p
                y = st["ybuf"]
                for m in range(KT):
                    msl = slice(m * 128, (m + 1) * 128)
                    nc.tensor.matmul(p[:, m, :], a_t[:, 3:5, msl],
                                     y[:, 3:5, _hsl(hf)], start=True,
                                     stop=False, perf_mode=DR)
                    nc.tensor.matmul(p[:, m, :], a_t[:, 5, msl],
                                     y[:, 5, _hsl(hf)], start=False,
                                     stop=True)

            def sry(st, hf):
                nc.vector.tensor_mul(out=st["ybuf"][:, 0:KT, _hsl(hf)],
                                     in0=st[f"p{hf}"],
                                     in1=st["thp"][:, :, _hsl(hf)])

            def srmm(st, hf):
                p, y = st[f"p{hf}"], st["ybuf"]
                for m in range(KT):
                    msl = slice(m * 128, (m + 1) * 128)
                    for j in range(KT):
                        nc.tensor.matmul(p[:, m, :],
                                         a_t[:, 2 * j:2 * j + 2, msl],
                                         y[:, 2 * j:2 * j + 2, _hsl(hf)],
                                         start=(j == 0), stop=(j == KT - 1),
                                         perf_mode=DR)

            def sfy(st, hf):
                if hf == 0:
                    st["t1"] = tp.tile([128, KT, GTOK], BF16, tag="t1",
                                       bufs=2, name=f"t1f{st['idx']}")
                nc.vector.tensor_mul(out=st["t1"][:, :, _hsl(hf)],
                                     in0=st[f"p{hf}"],
                                     in1=st["thp"][:, :, _hsl(hf)])

            def sfh(st, hf):
                # h stored half-major [p, hf, c, t] so the xbar DMA input
                # (c t) flattens contiguously.
                if hf == 0:
                    st["h_cm"] = gp.tile([128, 2, KT, HTOK], BF16, tag="hcm",
                                         name=f"hcm{st['idx']}")
                nc.vector.tensor_add(out=st["h_cm"][:, hf],
                                     in0=st["t1"][:, :, _hsl(hf)],
                                     in1=st["m2b"][:, :, _hsl(hf)])

            # ---------------- residual 1 (DMA transpose + Pool) -----------
            def dthh(st, hf):
                # h_tm layout [p, hf, c, it2, q]: one xbar DMA per half
                # ([128, 3*256] -> [128, 6, 128], chunks (c, it2) contiguous).
                if hf == 0:
                    st["h_tm"] = gp.tile([128, 2, KT, 2, 128], BF16,
                                         tag="htm", name=f"htm{st['idx']}")
                nc.scalar.dma_start_transpose(
                    st["h_tm"][:, hf].rearrange("p c i q -> p (c i) q"),
                    st["h_cm"][:, hf].rearrange("p c t -> p (c t)"))

            def rs(st, hf):
                if hf == 0:
                    st["x2_tm"] = x2p.tile([128, TPG, C], F32, tag="x2tm",
                                           name=f"x2tm{st['idx']}")
                isl = slice(2 * hf, 2 * hf + 2)
                x2r = st["x2_tm"][:, isl, :].rearrange(
                    "p i (c q) -> p i c q", c=KT)
                xr = st["x_tm"][:, isl, :].rearrange(
                    "p i (c q) -> p i c q", c=KT)
                htm = st["h_tm"][:, hf].rearrange("p c i q -> p i c q")
                nc.gpsimd.tensor_sub(out=x2r, in0=xr, in1=htm)

            # ---------------- MLP -----------------------------------------
            def mh(st, i):
                if i == 0:
                    st["hid"] = gp.tile([128, MH, GTOK], FP8, tag="hid",
                                        name=f"hid{st['idx']}")
                hsl = slice(i * 128, (i + 1) * 128)
                rhs = _rhs(st, 2)
                psh = ps.tile([128, GTOK], F32, tag="ps", name="psh")
                for j in range(2):
                    nc.tensor.matmul(psh, w1_t[:, 2 * j:2 * j + 2, hsl],
                                     rhs[:, j, :, :], start=(j == 0),
                                     stop=(j == 1), perf_mode=DR)
                nc.scalar.activation(out=st["hid"][:, i, :], in_=psh,
                                     func=AF.Gelu_apprx_tanh, scale=IWS,
                                     bias=b1_t[:, i:i + 1])

            def mo(st, it):
                tsl = slice(it * 128, (it + 1) * 128)
                pso = ps.tile([128, C], F32, tag="pso", name="pso", bufs=1)
                for j in range(MH // 2):
                    nc.tensor.matmul(pso,
                                     st["hid"][:, 2 * j:2 * j + 2, tsl],
                                     w2_t[:, 2 * j:2 * j + 2, :],
                                     start=(j == 0), stop=False,
                                     perf_mode=DR)
                nc.tensor.matmul(pso, ones_t, b2_t, start=False, stop=True)
                nc.vector.scalar_tensor_tensor(
                    out=st["x_tm"][:, it, :], in0=pso, scalar=IWS,
                    in1=st["x2_tm"][:, it, :], op0=OP.mult, op1=OP.add)
                row0 = (st["idx"] % NG) * GTOK + it * 128
                nc.scalar.dma_start(out=out_d[row0:row0 + 128, :],
                                    in_=st["x_tm"][:, it, :])

            # ---------------- conductor -----------------------------------
            ld(insts[0])
            load_late_weights()
            xn_alloc(insts[0], 1)
            stats(insts[0], 1)
            rsq(insts[0], 1)
            apply_ln(insts[0], 1)
            dmat_ln(insts[0], 1)
            if N > 1:
                ld(insts[1])
                xn_alloc(insts[1], 1)
            for m in range(KT):
                pjA(insts[0], m)
                pjB(insts[0], m)
            if N > 1:
                stats(insts[1], 1)
                rsq(insts[1], 1)
                apply_ln(insts[1], 1)
                dmat_ln(insts[1], 1)

            def rnd(S):
                for hf in range(2):
                    sry(S, hf)
                    srmm(S, hf)

            for k in range(N + 1):
                S = insts[k] if k < N else None
                P = insts[k + 1] if k + 1 < N else None
                Q = insts[k + 2] if k + 2 < N else None
                M = insts[k - 1] if 1 <= k else None

                if Q:
                    ld(Q)
                if M:
                    rs(M, 0)
                    rs(M, 1)
                    xn_alloc(M, 2)
                if Q:
                    xn_alloc(Q, 1)
                if P:
                    pjA(P, 0)
                if S:
                    sp1(S, 0)
                    sp1(S, 1)
                    rnd(S)          # round 1
                if P:
                    pjB(P, 0)
                if S:
                    rnd(S)          # round 2
                if Q:
                    stats(Q, 1)
                    rsq(Q, 1)
                if P:
                    pjA(P, 1)
                if S:
                    rnd(S)          # round 3
                if M:
                    stats(M, 2)
                    rsq(M, 2)
                if P:
                    pjA(P, 2)
                if S:
                    rnd(S)          # round 4
                if P:
                    pjB(P, 1)
                if Q:
                    apply_ln(Q, 1)
                    dmat_ln(Q, 1)
                if M:
                    apply_ln(M, 2)
                    dmat_ln(M, 2)
                if S:
                    rnd(S)          # round 5
                if P:
                    pjB(P, 2)
                if M:
                    for i in range(MH):
                        mh(M, i)
                if S:
                    # round 6 fused with the scan tail, half by half
                    sry(S, 0)
                    srmm(S, 0)
                    sfy(S, 0)
                    sfh(S, 0)
                    dthh(S, 0)
                    sry(S, 1)
                    srmm(S, 1)
                    sfy(S, 1)
                    sfh(S, 1)
                    dthh(S, 1)
                if M:
                    mo(M, 0)
                    mo(M, 1)
                    mo(M, 2)
                    mo(M, 3)
    return nc


_NC_CACHE = {}


def _get_nc():
    if "nc" not in _NC_CACHE:
        _NC_CACHE["nc"] = _patch_nc(build_nc())
    return _NC_CACHE["nc"]


def _perm_rows(w, bias_unused=None):
    """[C, D] -> [128, 4, D]: block (c,s) partition p = row 256c+2p+s."""
    D = w.shape[1]
    out = np.zeros((128, 4, D), np.float32)
    for c in range(2):
        for s in range(2):
            rows = 256 * c + 2 * np.arange(128) + s
            valid = rows < C
            out[valid, 2 * c + s, :] = w[rows[valid], :]
    return out


# ---------------------------------------------------------------- kernel --
def kernel(x, norm1_scale, norm1_bias, Wu, bu, Wg, bg, A,
           norm2_scale, norm2_bias, mlp_w1, mlp_b1, mlp_w2, mlp_b2,
           _return_raw=False):
    import ml_dtypes
    E4 = ml_dtypes.float8_e4m3
    f = np.float32
    x = np.asarray(x, f)
    norm1_scale = np.asarray(norm1_scale, f)
    norm1_bias = np.asarray(norm1_bias, f)
    Wu, bu = np.asarray(Wu, f), np.asarray(bu, f)
    Wg, bg = np.asarray(Wg, f), np.asarray(bg, f)
    A = np.asarray(A, f)
    norm2_scale = np.asarray(norm2_scale, f)
    norm2_bias = np.asarray(norm2_bias, f)
    mlp_w1, mlp_b1 = np.asarray(mlp_w1, f), np.asarray(mlp_b1, f)
    mlp_w2, mlp_b2 = np.asarray(mlp_w2, f), np.asarray(mlp_b2, f)

    # fold LN affine into downstream weights
    wu = norm1_scale[:, None] * Wu
    bu_f = bu + norm1_bias @ Wu
    wg = norm1_scale[:, None] * Wg
    bg_f = bg + norm1_bias @ Wg
    w1 = norm2_scale[:, None] * mlp_w1
    b1_f = mlp_b1 + norm2_bias @ mlp_w1

    wu_p = (_perm_rows(wu) * WS).astype(E4)
    wg_p = (_perm_rows(wg) * WS).astype(E4)
    w1_p = (_perm_rows(w1) * WS).astype(E4)

    a_p = np.zeros((128, 6, C), f)
    for s in range(6):
        a_p[:, s, :] = A[(s % 3) * 128:(s % 3) * 128 + 128, :]
    a_p = (a_p * WS).astype(E4)

    w2_p = np.zeros((128, MH, C), f)
    for s in range(MH):
        w2_p[:, s, :] = mlp_w2[s * 128:(s + 1) * 128, :]
    w2_p = (w2_p * WS).astype(E4)

    b2_p = (mlp_b2[None, :] * WS).astype(E4)
    ones_p = np.ones((1, 128), f).astype(E4)
    bu_p = np.ascontiguousarray((bu_f * WS).reshape(KT, 128).T)
    bg_p = np.ascontiguousarray((bg_f * 0.5).reshape(KT, 128).T)
    b1_p = np.ascontiguousarray(b1_f.reshape(MH, 128).T)

    cvals = np.array([EPS,
                      np.int32(1).view(f),
                      np.int32(MAGIC).view(f),
                      -0.5, 1.5, -1.0], f)
    cst_p = np.broadcast_to(cvals[None, :, None, None],
                            (128, 6, TPG, 1)).astype(f).copy()

    xs = x.reshape(NCORES, NTOK, C)
    in_maps = [{
        "x": np.ascontiguousarray(xs[i]),
        "wu": wu_p, "wg": wg_p, "a": a_p, "w1": w1_p, "w2": w2_p,
        "ones": ones_p, "b2": b2_p, "bu": bu_p, "bg": bg_p, "b1": b1_p,
        "cst": cst_p,
    } for i in range(NCORES)]

    res = run_bass_kernel_spmd(_get_nc(), in_maps, list(range(NCORES)))
    if _return_raw:
        return res
    out = np.concatenate([res.results[i]["out"] for i in range(NCORES)],
                         axis=0)
    return out.reshape(B, H, W, C).astype(np.float32)


# revision 28
# speedup vs baseline: 1.4558x; 1.2843x over previous
"""CSSM TinyViT block on 8 TRN2 NeuronCores — DMA-xbar-transpose version.

Strategy
--------
Data-parallel over batch: B=16 -> 2 samples (2048 tokens) per core, processed
as 4 groups of 512 tokens in a software pipeline.

All channel-mixing matmuls run fp8(e4m3) DoubleRow.  Token-major -> channel-
major layout changes go through the DMA crossbar transpose (u16 views of fp8
pairs; the channel order becomes (256c + 2p + s), which is folded into the
weight row order host-side).  h comes back token-major the same way in bf16.
This removes every PE transpose and every PSUM->SBUF copy of the baseline.

Biases ride the epilogues: bg via the tanh ACT bias AP, bu via the m2 STT
scalar AP, b1 via the gelu ACT bias AP, b2 via the ones-row matmul.

Engine placement: DVE keeps only PSUM-coupled elementwise work (scan gate
multiplies, m2, m_out epilogue, bn_stats); the LN rsqrt chains (rsqrt bit
hack + 1 Newton step), the residual subtract and the xn pad memsets run on
Pool; LN applies, tanh/gelu and the m2->fp8 copies run on ACT.

The emission order interleaves PE-decoupled DVE filler (proj epilogues,
bn_stats of neighbour groups, m_out) between the scan rounds so the DVE never
waits on the scan matmuls.

PSUM: scan p [128,3,512] f32 (3 banks) + 4-deep 1-bank ring (proj/MLP) = 7.
"""
import json
import os
import types

import numpy as np

import concourse.bass as bass
import concourse.mybir as mybir
from concourse.tile import TileContext
from concourse.bass_utils import run_bass_kernel_spmd

F32 = mybir.dt.float32
FP8 = mybir.dt.float8e4
BF16 = mybir.dt.bfloat16
U16 = mybir.dt.uint16
I32 = mybir.dt.int32
AF = mybir.ActivationFunctionType
OP = mybir.AluOpType
DR = mybir.MatmulPerfMode.DoubleRow

B, H, W, C, T = 16, 32, 32, 384, 8
HID = 4 * C
EPS = 1e-6
NCORES = 8
BSH = B // NCORES
NTOK = BSH * H * W             # 2048 tokens per core
GTOK = 512                     # tokens per group
NG = NTOK // GTOK              # 4 groups
TPG = GTOK // 128              # 4 token-tiles per group
KT = C // 128                  # 3 channel tiles
MH = HID // 128                # 12 hidden tiles
WS = 64.0                      # fp8 weight scale
SH = 0.5 / WS                  # dequant incl. the tanh 0.5
IWS = 1.0 / WS
MAGIC = 0x5F3759DF
CPAD = 512                     # xn padded channel count (u16-transposable)


# ---------------------------------------------------------------- bir fix --
# This container's walrus rejects instructions whose sync-wait list exceeds
# the opcode's wait slots.  Move excess waits onto EventSemaphore
# instructions inserted before the instruction on the same engine queue.
_WAIT_LIMITS = {"Drain": 0, "DmaTransposeAnt": 0}
_WAIT_DEFAULT = 1


def _fix_bir_json(bj: bytes) -> bytes:
    bir = json.loads(bj)
    counter = [0]

    def fix_blocks(blocks):
        for b in blocks:
            insts = b.get("instructions")
            if insts:
                new = []
                for inst in insts:
                    si = inst.get("sync_info")
                    waits = (si or {}).get("on_wait") or []
                    limit = _WAIT_LIMITS.get(inst.get("opcode"), _WAIT_DEFAULT)
                    if len(waits) > limit:
                        n_extra = len(waits) - limit
                        extra, keep = waits[:n_extra], waits[n_extra:]
                        for wv in extra:
                            counter[0] += 1
                            new.append({
                                "name": f"I-wfix-{counter[0]}",
                                "opcode": "EventSemaphore",
                                "engine": inst["engine"],
                                "ins": [],
                                "outs": [],
                                "sync_info": {"on_update": [], "on_wait": [wv]},
                                "debug": inst.get("debug", 0),
                            })
                        si["on_wait"] = keep
                    new.append(inst)
                b["instructions"] = new
            fix_blocks(b.get("blocks") or [])

    for fn in bir.get("functions", []):
        fix_blocks(fn.get("blocks") or [])
    return json.dumps(bir).encode()


def _patch_nc(nc):
    orig = nc.to_json_bytes

    def to_json_bytes(self):
        return _fix_bir_json(orig())

    nc.to_json_bytes = types.MethodType(to_json_bytes, nc)
    return nc


# ----------------------------------------------------------- device build --
def build_nc(repeat=1):
    nc = bass.Bass()

    x_in = nc.declare_dram_parameter("x", [NTOK, C], F32, isOutput=False)
    wu_d = nc.declare_dram_parameter("wu", [128, 4, C], FP8, isOutput=False)
    wg_d = nc.declare_dram_parameter("wg", [128, 4, C], FP8, isOutput=False)
    a_d = nc.declare_dram_parameter("a", [128, 6, C], FP8, isOutput=False)
    w1_d = nc.declare_dram_parameter("w1", [128, 4, HID], FP8, isOutput=False)
    w2_d = nc.declare_dram_parameter("w2", [128, MH, C], FP8, isOutput=False)
    ones_d = nc.declare_dram_parameter("ones", [1, 128], FP8, isOutput=False)
    b2_d = nc.declare_dram_parameter("b2", [1, C], FP8, isOutput=False)
    bu_d = nc.declare_dram_parameter("bu", [128, KT], F32, isOutput=False)
    bg_d = nc.declare_dram_parameter("bg", [128, KT], F32, isOutput=False)
    b1_d = nc.declare_dram_parameter("b1", [128, MH], F32, isOutput=False)
    cst_d = nc.declare_dram_parameter("cst", [128, 6, TPG, 1], F32,
                                      isOutput=False)
    out_d = nc.declare_dram_parameter("out", [NTOK, C], F32, isOutput=True)

    N = NG * repeat

    with TileContext(nc) as tc:
        with (
            tc.tile_pool(name="wp", bufs=1) as wp,
            tc.tile_pool(name="xp", bufs=4) as xp,
            tc.tile_pool(name="x2p", bufs=2) as x2p,
            tc.tile_pool(name="gp", bufs=2) as gp,
            tc.tile_pool(name="tp", bufs=2) as tp,
            tc.tile_pool(name="sp", bufs=2) as sp,
            tc.tile_pool(name="ps", bufs=3, space="PSUM") as ps,
            tc.tile_pool(name="scanp", bufs=2, space="PSUM") as scanp,
        ):
            # ---- weights / constants ----
            wu_t = wp.tile([128, 4, C], FP8, tag="wu")
            wg_t = wp.tile([128, 4, C], FP8, tag="wg")
            a_t = wp.tile([128, 6, C], FP8, tag="a")
            w1_t = wp.tile([128, 4, HID], FP8, tag="w1")
            w2_t = wp.tile([128, MH, C], FP8, tag="w2")
            ones_t = wp.tile([1, 128], FP8, tag="ones")
            b2_t = wp.tile([1, C], FP8, tag="b2")
            bu_t = wp.tile([128, KT], F32, tag="bu")
            bg_t = wp.tile([128, KT], F32, tag="bg")
            b1_t = wp.tile([128, MH], F32, tag="b1")
            cst_t = wp.tile([128, 6, TPG, 1], F32, tag="cst")
            nc.sync.dma_start(out=cst_t, in_=cst_d[:, :, :, :])
            nc.sync.dma_start(out=wu_t, in_=wu_d[:, :, :])
            nc.sync.dma_start(out=wg_t, in_=wg_d[:, :, :])
            nc.sync.dma_start(out=a_t, in_=a_d[:, :, :])
            nc.sync.dma_start(out=bu_t, in_=bu_d[:, :])
            nc.sync.dma_start(out=bg_t, in_=bg_d[:, :])
            nc.sync.dma_start(out=b1_t, in_=b1_d[:, :])

            def load_late_weights():
                nc.sync.dma_start(out=w1_t, in_=w1_d[:, :, :])
                nc.sync.dma_start(out=w2_t, in_=w2_d[:, :, :])
                nc.sync.dma_start(out=ones_t, in_=ones_d[:, :])
                nc.sync.dma_start(out=b2_t, in_=b2_d[:, :])

            insts = [dict(idx=i) for i in range(N)]

            # ---------------- load + LN1 stats -----------------------------
            def ld(st):
                st["x_tm"] = xp.tile([128, TPG, C], F32, tag="xtm",
                                     name=f"xtm{st['idx']}")
                row0 = (st["idx"] % NG) * GTOK
                for it in range(TPG):
                    r = row0 + it * 128
                    nc.sync.dma_start(out=st["x_tm"][:, it, :],
                                      in_=x_in[r:r + 128, :])

            def stats(st, which):
                src = st["x_tm"] if which == 1 else st["x2_tm"]
                mvb = sp.tile([128, TPG, 2], F32, tag=f"mvb{which}",
                              name=f"mvb{which}{st['idx']}")
                for it in range(TPG):
                    st6 = sp.tile([128, 6], F32, tag=f"st6{which}", bufs=2)
                    nc.vector.bn_stats(out=st6, in_=src[:, it, :])
                    nc.vector.bn_aggr(out=mvb[:, it, :], in_=st6)
                st[f"mvb{which}"] = mvb

            def rsq(st, which):
                """Pool: r1 = rsqrt(var+eps), mnr = -mean*r1 (2 Newton).

                Pool only passes the ISA check for TensorTensor/TensorCopy/
                Memset, so every scalar rides a broadcast constant tile
                (cst_t[:, j] = [128, TPG, 1]): 0=eps 1=int(1) 2=magic
                3=-0.5 4=1.5 5=-1.
                """
                mvb = st[f"mvb{which}"]
                S = [128, TPG, 1]
                tg = f"rq{which}"
                TT = nc.gpsimd.tensor_tensor
                # eps-add + bit hack on DVE (Pool shifts need i64 out);
                # the Newton steps run on Pool.
                vpe = sp.tile(S, F32, tag=f"vpe{tg}")
                nc.vector.tensor_scalar(out=vpe, in0=mvb[:, :, 1:2],
                                        scalar1=EPS, scalar2=None, op0=OP.add)
                i1 = sp.tile(S, I32, tag=f"i1{tg}")
                nc.vector.tensor_scalar(out=i1, in0=vpe.bitcast(I32),
                                        scalar1=1, scalar2=None,
                                        op0=OP.logical_shift_right)
                i2 = sp.tile(S, I32, tag=f"i2{tg}")
                nc.vector.tensor_scalar(out=i2, in0=i1, scalar1=MAGIC,
                                        scalar2=-1, op0=OP.subtract,
                                        op1=OP.mult)
                r0 = i2.bitcast(F32)
                t = sp.tile(S, F32, tag=f"t{tg}")
                TT(out=t, in0=vpe, in1=r0, op=OP.mult)
                TT(out=t, in0=t, in1=r0, op=OP.mult)
                TT(out=t, in0=t, in1=cst_t[:, 3], op=OP.mult)
                TT(out=t, in0=t, in1=cst_t[:, 4], op=OP.add)
                r1 = sp.tile(S, F32, tag=f"r1{tg}",
                             name=f"r1{tg}{st['idx']}")
                TT(out=r1, in0=t, in1=r0, op=OP.mult)
                t2 = sp.tile(S, F32, tag=f"t2{tg}")
                TT(out=t2, in0=vpe, in1=r1, op=OP.mult)
                TT(out=t2, in0=t2, in1=r1, op=OP.mult)
                TT(out=t2, in0=t2, in1=cst_t[:, 3], op=OP.mult)
                TT(out=t2, in0=t2, in1=cst_t[:, 4], op=OP.add)
                rb = sp.tile(S, F32, tag=f"rb{tg}",
                             name=f"rb{tg}{st['idx']}")
                TT(out=rb, in0=t2, in1=r1, op=OP.mult)
                mm = sp.tile(S, F32, tag=f"mm{tg}")
                TT(out=mm, in0=mvb[:, :, 0:1], in1=rb, op=OP.mult)
                mnr = sp.tile(S, F32, tag=f"mnr{tg}",
                              name=f"mnr{tg}{st['idx']}")
                TT(out=mnr, in0=mm, in1=cst_t[:, 5], op=OP.mult)
                st[f"r{which}"] = rb
                st[f"mnr{which}"] = mnr

            # ---------------- LN apply + DMA transpose ---------------------
            def xn_alloc(st, which):
                xn = gp.tile([128, TPG, CPAD], FP8, tag=f"xn{which}",
                             name=f"xn{which}_{st['idx']}")
                nc.gpsimd.memset(xn[:, :, C:CPAD], 0.0)
                st[f"xn{which}"] = xn

            def apply_ln(st, which):
                src = st["x_tm"] if which == 1 else st["x2_tm"]
                xn = st[f"xn{which}"]
                r1, mnr = st[f"r{which}"], st[f"mnr{which}"]
                for it in range(TPG):
                    nc.scalar.activation(out=xn[:, it, 0:C],
                                         in_=src[:, it, :],
                                         func=AF.Identity,
                                         scale=r1[:, it, :],
                                         bias=mnr[:, it, :])

            def dmat_ln(st, which):
                xn16 = st[f"xn{which}"].bitcast(U16)
                xcm = gp.tile([128, 2, GTOK], U16, tag=f"xcm{which}",
                              name=f"xcm{which}_{st['idx']}")
                for it in range(TPG):
                    nc.scalar.dma_start_transpose(
                        xcm[:, :, it * 128:(it + 1) * 128], xn16[:, it, :])
                st[f"xcm{which}"] = xcm

            def _rhs(st, which):
                """[128, c, 2, 512] fp8 view: (sub s, token t) per chunk."""
                x8 = st[f"xcm{which}"].bitcast(FP8)
                return x8.rearrange("p c (t s) -> p c s t", s=2)

            # ---------------- projections ---------------------------------
            def pjA(st, m):
                """PE psu/psg matmuls + ACT tanh for m-tile."""
                if m == 0:
                    st["ybuf"] = gp.tile([128, 6, GTOK], FP8, tag="ybuf",
                                         name=f"ybuf{st['idx']}")
                    st["thp"] = gp.tile([128, KT, GTOK], BF16, tag="thp",
                                        name=f"thp{st['idx']}")
                    st["m2b"] = gp.tile([128, KT, GTOK], BF16, tag="m2b",
                                        name=f"m2b{st['idx']}")
                msl = slice(m * 128, (m + 1) * 128)
                rhs = _rhs(st, 1)
                psu = ps.tile([128, GTOK], F32, tag="ps", name="psu")
                psg = ps.tile([128, GTOK], F32, tag="ps", name="psg")
                for j in range(2):
                    nc.tensor.matmul(psu, wu_t[:, 2 * j:2 * j + 2, msl],
                                     rhs[:, j, :, :], start=(j == 0),
                                     stop=(j == 1), perf_mode=DR)
                for j in range(2):
                    nc.tensor.matmul(psg, wg_t[:, 2 * j:2 * j + 2, msl],
                                     rhs[:, j, :, :], start=(j == 0),
                                     stop=(j == 1), perf_mode=DR)
                thraw = tp.tile([128, GTOK], BF16, tag="thraw", bufs=2,
                                name=f"thraw{st['idx']}_{m}")
                nc.scalar.activation(out=thraw, in_=psg, func=AF.Tanh,
                                     scale=SH, bias=bg_t[:, m:m + 1])
                st[f"thraw{m}"] = thraw
                st[f"psu{m}"] = psu

            def pjB(st, m):
                """DVE thp/thm/m2 + ACT ybuf copy for m-tile."""
                thraw, psu = st[f"thraw{m}"], st[f"psu{m}"]
                nc.vector.tensor_scalar(out=st["thp"][:, m, :], in0=thraw,
                                        scalar1=1.0, scalar2=SH,
                                        op0=OP.add, op1=OP.mult)
                thm = tp.tile([128, GTOK], BF16, tag="thm", bufs=2,
                              name=f"thm{st['idx']}_{m}")
                nc.vector.tensor_scalar(out=thm, in0=thraw,
                                        scalar1=-1.0, scalar2=SH,
                                        op0=OP.add, op1=OP.mult)
                nc.vector.scalar_tensor_tensor(
                    out=st["m2b"][:, m, :], in0=psu, scalar=bu_t[:, m:m + 1],
                    in1=thm, op0=OP.add, op1=OP.mult)
                nc.scalar.activation(out=st["ybuf"][:, KT + m, :],
                                     in_=st["m2b"][:, m, :], func=AF.Copy)

            # ---------------- scan (two independent half-token chains) ----
            HTOK = GTOK // 2

            def _hsl(hf):
                return slice(hf * HTOK, (hf + 1) * HTOK)

            def sp1(st, hf):
                p = scanp.tile([128, KT, HTOK], F32, tag="p",
                               name=f"scp{hf}")
                st[f"p{hf}"] = p
                y = st["ybuf"]
                for m in range(KT):
                    msl = slice(m * 128, (m + 1) * 128)
                    nc.tensor.matmul(p[:, m, :], a_t[:, 3:5, msl],
                                     y[:, 3:5, _hsl(hf)], start=True,
                                     stop=False, perf_mode=DR)
                    nc.tensor.matmul(p[:, m, :], a_t[:, 5, msl],
                                     y[:, 5, _hsl(hf)], start=False,
                                     stop=True)

            def sry(st, hf):
                nc.vector.tensor_mul(out=st["ybuf"][:, 0:KT, _hsl(hf)],
                                     in0=st[f"p{hf}"],
                                     in1=st["thp"][:, :, _hsl(hf)])

            def srmm(st, hf):
                p, y = st[f"p{hf}"], st["ybuf"]
                for m in range(KT):
                    msl = slice(m * 128, (m + 1) * 128)
                    for j in range(KT):
                        nc.tensor.matmul(p[:, m, :],
                                         a_t[:, 2 * j:2 * j + 2, msl],
                                         y[:, 2 * j:2 * j + 2, _hsl(hf)],
                                         start=(j == 0), stop=(j == KT - 1),
                                         perf_mode=DR)

            def sfy(st, hf):
                if hf == 0:
                    st["t1"] = tp.tile([128, KT, GTOK], BF16, tag="t1",
                                       bufs=2, name=f"t1f{st['idx']}")
                nc.vector.tensor_mul(out=st["t1"][:, :, _hsl(hf)],
                                     in0=st[f"p{hf}"],
                                     in1=st["thp"][:, :, _hsl(hf)])

            def sfh(st, hf):
                # h stored half-major [p, hf, c, t] so the xbar DMA input
                # (c t) flattens contiguously.
                if hf == 0:
                    st["h_cm"] = gp.tile([128, 2, KT, HTOK], BF16, tag="hcm",
                                         name=f"hcm{st['idx']}")
                nc.vector.tensor_add(out=st["h_cm"][:, hf],
                                     in0=st["t1"][:, :, _hsl(hf)],
                                     in1=st["m2b"][:, :, _hsl(hf)])

            # ---------------- residual 1 (DMA transpose + Pool) -----------
            def dthh(st, hf):
                # h_tm layout [p, hf, c, it2, q]: one xbar DMA per half
                # ([128, 3*256] -> [128, 6, 128], chunks (c, it2) contiguous).
                if hf == 0:
                    st["h_tm"] = gp.tile([128, 2, KT, 2, 128], BF16,
                                         tag="htm", name=f"htm{st['idx']}")
                nc.scalar.dma_start_transpose(
                    st["h_tm"][:, hf].rearrange("p c i q -> p (c i) q"),
                    st["h_cm"][:, hf].rearrange("p c t -> p (c t)"))

            def rs(st, hf):
                if hf == 0:
                    st["x2_tm"] = x2p.tile([128, TPG, C], F32, tag="x2tm",
                                           name=f"x2tm{st['idx']}")
                isl = slice(2 * hf, 2 * hf + 2)
                x2r = st["x2_tm"][:, isl, :].rearrange(
                    "p i (c q) -> p i c q", c=KT)
                xr = st["x_tm"][:, isl, :].rearrange(
                    "p i (c q) -> p i c q", c=KT)
                htm = st["h_tm"][:, hf].rearrange("p c i q -> p i c q")
                nc.gpsimd.tensor_sub(out=x2r, in0=xr, in1=htm)

            # ---------------- MLP -----------------------------------------
            def mh(st, i):
                if i == 0:
                    st["hid"] = gp.tile([128, MH, GTOK], FP8, tag="hid",
                                        name=f"hid{st['idx']}")
                hsl = slice(i * 128, (i + 1) * 128)
                rhs = _rhs(st, 2)
                psh = ps.tile([128, GTOK], F32, tag="ps", name="psh")
                for j in range(2):
                    nc.tensor.matmul(psh, w1_t[:, 2 * j:2 * j + 2, hsl],
                                     rhs[:, j, :, :], start=(j == 0),
                                     stop=(j == 1), perf_mode=DR)
                nc.scalar.activation(out=st["hid"][:, i, :], in_=psh,
                                     func=AF.Gelu_apprx_tanh, scale=IWS,
                                     bias=b1_t[:, i:i + 1])

            def mo(st, it):
                if it == 0:
                    st["ob"] = x2p.tile([128, TPG, C], F32, tag="ob",
                                        name=f"ob{st['idx']}")
                tsl = slice(it * 128, (it + 1) * 128)
                pso = ps.tile([128, C], F32, tag="pso", name="pso", bufs=1)
                for j in range(MH // 2):
                    nc.tensor.matmul(pso,
                                     st["hid"][:, 2 * j:2 * j + 2, tsl],
                                     w2_t[:, 2 * j:2 * j + 2, :],
                                     start=(j == 0), stop=False,
                                     perf_mode=DR)
                nc.tensor.matmul(pso, ones_t, b2_t, start=False, stop=True)
                tmo = tp.tile([128, C], F32, tag="tmo", bufs=2,
                              name=f"tmo{st['idx']}_{it}")
                nc.scalar.activation(out=tmo, in_=pso, func=AF.Copy,
                                     scale=IWS)
                nc.gpsimd.tensor_add(out=st["ob"][:, it, :], in0=tmo,
                                     in1=st["x2_tm"][:, it, :])
                row0 = (st["idx"] % NG) * GTOK + it * 128
                nc.scalar.dma_start(out=out_d[row0:row0 + 128, :],
                                    in_=st["ob"][:, it, :])

            # ---------------- conductor -----------------------------------
            ld(insts[0])
            load_late_weights()
            xn_alloc(insts[0], 1)
            stats(insts[0], 1)
            rsq(insts[0], 1)
            apply_ln(insts[0], 1)
            dmat_ln(insts[0], 1)
            if N > 1:
                ld(insts[1])
                xn_alloc(insts[1], 1)
            for m in range(KT):
                pjA(insts[0], m)
                pjB(insts[0], m)
            if N > 1:
                stats(insts[1], 1)
                rsq(insts[1], 1)
                apply_ln(insts[1], 1)
                dmat_ln(insts[1], 1)

            def rnd(S):
                for hf in range(2):
                    sry(S, hf)
                    srmm(S, hf)

            for k in range(N + 1):
                S = insts[k] if k < N else None
                P = insts[k + 1] if k + 1 < N else None
                Q = insts[k + 2] if k + 2 < N else None
                M = insts[k - 1] if 1 <= k else None

                if Q:
                    ld(Q)
                if M:
                    rs(M, 0)
                    rs(M, 1)
                    xn_alloc(M, 2)
                if Q:
                    xn_alloc(Q, 1)
                if P:
                    pjA(P, 0)
                if S:
                    sp1(S, 0)
                    sp1(S, 1)
                    rnd(S)          # round 1
                if P:
                    pjB(P, 0)
                if S:
                    rnd(S)          # round 2
                if Q:
                    stats(Q, 1)
                    rsq(Q, 1)
                if P:
                    pjA(P, 1)
                if S:
                    rnd(S)          # round 3
                if M:
                    stats(M, 2)
                    rsq(M, 2)
                if P:
                    pjA(P, 2)
                if S:
                    rnd(S)          # round 4
                if P:
                    pjB(P, 1)
                if Q:
                    apply_ln(Q, 1)
                    dmat_ln(Q, 1)
                if M:
                    apply_ln(M, 2)
                    dmat_ln(M, 2)
                if S:
                    rnd(S)          # round 5
                if P:
                    pjB(P, 2)
                if M:
                    for i in range(MH):
                        mh(M, i)
                if S:
                    # round 6 fused with the scan tail, half by half
                    sry(S, 0)
                    srmm(S, 0)
                    sfy(S, 0)
                    sfh(S, 0)
                    dthh(S, 0)
                    sry(S, 1)
                    srmm(S, 1)
                    sfy(S, 1)
                    sfh(S, 1)
                    dthh(S, 1)
                if M:
                    mo(M, 0)
                    mo(M, 1)
                    mo(M, 2)
                    mo(M, 3)
    return nc


_NC_CACHE = {}


def _get_nc():
    if "nc" not in _NC_CACHE:
        _NC_CACHE["nc"] = _patch_nc(build_nc())
    return _NC_CACHE["nc"]


def _perm_rows(w, bias_unused=None):
    """[C, D] -> [128, 4, D]: block (c,s) partition p = row 256c+2p+s."""
    D = w.shape[1]
    out = np.zeros((128, 4, D), np.float32)
    for c in range(2):
        for s in range(2):
            rows = 256 * c + 2 * np.arange(128) + s
            valid = rows < C
            out[valid, 2 * c + s, :] = w[rows[valid], :]
    return out


# ---------------------------------------------------------------- kernel --
def kernel(x, norm1_scale, norm1_bias, Wu, bu, Wg, bg, A,
           norm2_scale, norm2_bias, mlp_w1, mlp_b1, mlp_w2, mlp_b2,
           _return_raw=False):
    import ml_dtypes
    E4 = ml_dtypes.float8_e4m3
    f = np.float32
    x = np.asarray(x, f)
    norm1_scale = np.asarray(norm1_scale, f)
    norm1_bias = np.asarray(norm1_bias, f)
    Wu, bu = np.asarray(Wu, f), np.asarray(bu, f)
    Wg, bg = np.asarray(Wg, f), np.asarray(bg, f)
    A = np.asarray(A, f)
    norm2_scale = np.asarray(norm2_scale, f)
    norm2_bias = np.asarray(norm2_bias, f)
    mlp_w1, mlp_b1 = np.asarray(mlp_w1, f), np.asarray(mlp_b1, f)
    mlp_w2, mlp_b2 = np.asarray(mlp_w2, f), np.asarray(mlp_b2, f)

    # fold LN affine into downstream weights
    wu = norm1_scale[:, None] * Wu
    bu_f = bu + norm1_bias @ Wu
    wg = norm1_scale[:, None] * Wg
    bg_f = bg + norm1_bias @ Wg
    w1 = norm2_scale[:, None] * mlp_w1
    b1_f = mlp_b1 + norm2_bias @ mlp_w1

    wu_p = (_perm_rows(wu) * WS).astype(E4)
    wg_p = (_perm_rows(wg) * WS).astype(E4)
    w1_p = (_perm_rows(w1) * WS).astype(E4)

    a_p = np.zeros((128, 6, C), f)
    for s in range(6):
        a_p[:, s, :] = A[(s % 3) * 128:(s % 3) * 128 + 128, :]
    a_p = (a_p * WS).astype(E4)

    w2_p = np.zeros((128, MH, C), f)
    for s in range(MH):
        w2_p[:, s, :] = mlp_w2[s * 128:(s + 1) * 128, :]
    w2_p = (w2_p * WS).astype(E4)

    b2_p = (mlp_b2[None, :] * WS).astype(E4)
    ones_p = np.ones((1, 128), f).astype(E4)
    bu_p = np.ascontiguousarray((bu_f * WS).reshape(KT, 128).T)
    bg_p = np.ascontiguousarray((bg_f * 0.5).reshape(KT, 128).T)
    b1_p = np.ascontiguousarray(b1_f.reshape(MH, 128).T)

    cvals = np.array([EPS,
                      np.int32(1).view(f),
                      np.int32(MAGIC).view(f),
                      -0.5, 1.5, -1.0], f)
    cst_p = np.broadcast_to(cvals[None, :, None, None],
                            (128, 6, TPG, 1)).astype(f).copy()

    xs = x.reshape(NCORES, NTOK, C)
    in_maps = [{
        "x": np.ascontiguousarray(xs[i]),
        "wu": wu_p, "wg": wg_p, "a": a_p, "w1": w1_p, "w2": w2_p,
        "ones": ones_p, "b2": b2_p, "bu": bu_p, "bg": bg_p, "b1": b1_p,
        "cst": cst_p,
    } for i in range(NCORES)]

    res = run_bass_kernel_spmd(_get_nc(), in_maps, list(range(NCORES)))
    if _return_raw:
        return res
    out = np.concatenate([res.results[i]["out"] for i in range(NCORES)],
                         axis=0)
    return out.reshape(B, H, W, C).astype(np.float32)


# revision 31
# speedup vs baseline: 1.6459x; 1.1306x over previous
"""CSSM TinyViT block on 8 TRN2 NeuronCores — DMA-xbar-transpose version.

Strategy
--------
Data-parallel over batch: B=16 -> 2 samples (2048 tokens) per core, processed
as 4 groups of 512 tokens in a software pipeline.

All channel-mixing matmuls run fp8(e4m3) DoubleRow.  Token-major -> channel-
major layout changes go through the DMA crossbar transpose (u16 views of fp8
pairs; the channel order becomes (256c + 2p + s), which is folded into the
weight row order host-side).  h comes back token-major the same way in bf16.
This removes every PE transpose and every PSUM->SBUF copy of the baseline.

Biases ride the epilogues: bg via the tanh ACT bias AP, bu via the m2 STT
scalar AP, b1 via the gelu ACT bias AP, b2 via the ones-row matmul.

Engine placement: DVE keeps only PSUM-coupled elementwise work (scan gate
multiplies, m2, m_out epilogue, bn_stats); the LN rsqrt chains (rsqrt bit
hack + 1 Newton step), the residual subtract and the xn pad memsets run on
Pool; LN applies, tanh/gelu and the m2->fp8 copies run on ACT.

The emission order interleaves PE-decoupled DVE filler (proj epilogues,
bn_stats of neighbour groups, m_out) between the scan rounds so the DVE never
waits on the scan matmuls.

PSUM: scan p [128,3,512] f32 (3 banks) + 4-deep 1-bank ring (proj/MLP) = 7.
"""
import json
import os
import types

import numpy as np

import concourse.bass as bass
import concourse.mybir as mybir
from concourse.tile import TileContext
from concourse.bass_utils import run_bass_kernel_spmd

F32 = mybir.dt.float32
FP8 = mybir.dt.float8e4
BF16 = mybir.dt.bfloat16
U16 = mybir.dt.uint16
I32 = mybir.dt.int32
AF = mybir.ActivationFunctionType
OP = mybir.AluOpType
DR = mybir.MatmulPerfMode.DoubleRow

B, H, W, C, T = 16, 32, 32, 384, 8
HID = 4 * C
EPS = 1e-6
NCORES = 8
BSH = B // NCORES
NTOK = BSH * H * W             # 2048 tokens per core
GTOK = 512                     # tokens per group
NG = NTOK // GTOK              # 4 groups
TPG = GTOK // 128              # 4 token-tiles per group
KT = C // 128                  # 3 channel tiles
MH = HID // 128                # 12 hidden tiles
WS = 64.0                      # fp8 weight scale
SH = 0.5 / WS                  # dequant incl. the tanh 0.5
IWS = 1.0 / WS
MAGIC = 0x5F3759DF
CPAD = 512                     # xn padded channel count (u16-transposable)


# ---------------------------------------------------------------- bir fix --
# This container's walrus rejects instructions whose sync-wait list exceeds
# the opcode's wait slots.  Move excess waits onto EventSemaphore
# instructions inserted before the instruction on the same engine queue.
_WAIT_LIMITS = {"Drain": 0, "DmaTransposeAnt": 0}
_WAIT_DEFAULT = 1


def _fix_bir_json(bj: bytes) -> bytes:
    bir = json.loads(bj)
    counter = [0]

    def fix_blocks(blocks):
        for b in blocks:
            insts = b.get("instructions")
            if insts:
                new = []
                for inst in insts:
                    si = inst.get("sync_info")
                    waits = (si or {}).get("on_wait") or []
                    limit = _WAIT_LIMITS.get(inst.get("opcode"), _WAIT_DEFAULT)
                    if len(waits) > limit:
                        n_extra = len(waits) - limit
                        extra, keep = waits[:n_extra], waits[n_extra:]
                        for wv in extra:
                            counter[0] += 1
                            new.append({
                                "name": f"I-wfix-{counter[0]}",
                                "opcode": "EventSemaphore",
                                "engine": inst["engine"],
                                "ins": [],
                                "outs": [],
                                "sync_info": {"on_update": [], "on_wait": [wv]},
                                "debug": inst.get("debug", 0),
                            })
                        si["on_wait"] = keep
                    new.append(inst)
                b["instructions"] = new
            fix_blocks(b.get("blocks") or [])

    for fn in bir.get("functions", []):
        fix_blocks(fn.get("blocks") or [])
    return json.dumps(bir).encode()


def _patch_nc(nc):
    orig = nc.to_json_bytes

    def to_json_bytes(self):
        return _fix_bir_json(orig())

    nc.to_json_bytes = types.MethodType(to_json_bytes, nc)
    return nc


# ----------------------------------------------------------- device build --
def build_nc(repeat=1):
    nc = bass.Bass()

    x_in = nc.declare_dram_parameter("x", [NTOK, C], F32, isOutput=False)
    wu_d = nc.declare_dram_parameter("wu", [128, 4, C], FP8, isOutput=False)
    wg_d = nc.declare_dram_parameter("wg", [128, 4, C], FP8, isOutput=False)
    a_d = nc.declare_dram_parameter("a", [128, 6, C], FP8, isOutput=False)
    w1_d = nc.declare_dram_parameter("w1", [128, 4, HID], FP8, isOutput=False)
    w2_d = nc.declare_dram_parameter("w2", [128, MH, C], FP8, isOutput=False)
    ones_d = nc.declare_dram_parameter("ones", [1, 128], FP8, isOutput=False)
    b2_d = nc.declare_dram_parameter("b2", [1, C], FP8, isOutput=False)
    bu_d = nc.declare_dram_parameter("bu", [128, KT], F32, isOutput=False)
    bg_d = nc.declare_dram_parameter("bg", [128, KT], F32, isOutput=False)
    b1_d = nc.declare_dram_parameter("b1", [128, MH], F32, isOutput=False)
    cst_d = nc.declare_dram_parameter("cst", [128, 6, TPG, 1], F32,
                                      isOutput=False)
    out_d = nc.declare_dram_parameter("out", [NTOK, C], F32, isOutput=True)

    N = NG * repeat

    with TileContext(nc) as tc:
        with (
            tc.tile_pool(name="wp", bufs=1) as wp,
            tc.tile_pool(name="xp", bufs=4) as xp,
            tc.tile_pool(name="x2p", bufs=2) as x2p,
            tc.tile_pool(name="gp", bufs=2) as gp,
            tc.tile_pool(name="tp", bufs=2) as tp,
            tc.tile_pool(name="sp", bufs=2) as sp,
            tc.tile_pool(name="ps", bufs=3, space="PSUM") as ps,
            tc.tile_pool(name="scanp", bufs=2, space="PSUM") as scanp,
        ):
            # ---- weights / constants ----
            wu_t = wp.tile([128, 4, C], FP8, tag="wu")
            wg_t = wp.tile([128, 4, C], FP8, tag="wg")
            a_t = wp.tile([128, 6, C], FP8, tag="a")
            w1_t = wp.tile([128, 4, HID], FP8, tag="w1")
            w2_t = wp.tile([128, MH, C], FP8, tag="w2")
            ones_t = wp.tile([1, 128], FP8, tag="ones")
            b2_t = wp.tile([1, C], FP8, tag="b2")
            bu_t = wp.tile([128, KT], F32, tag="bu")
            bg_t = wp.tile([128, KT], F32, tag="bg")
            b1_t = wp.tile([128, MH], F32, tag="b1")
            cst_t = wp.tile([128, 6, TPG, 1], F32, tag="cst")
            nc.sync.dma_start(out=cst_t, in_=cst_d[:, :, :, :])
            nc.sync.dma_start(out=wu_t, in_=wu_d[:, :, :])
            nc.sync.dma_start(out=wg_t, in_=wg_d[:, :, :])
            nc.sync.dma_start(out=a_t, in_=a_d[:, :, :])
            nc.sync.dma_start(out=bu_t, in_=bu_d[:, :])
            nc.sync.dma_start(out=bg_t, in_=bg_d[:, :])
            nc.sync.dma_start(out=b1_t, in_=b1_d[:, :])

            def load_late_weights():
                nc.sync.dma_start(out=w1_t, in_=w1_d[:, :, :])
                nc.sync.dma_start(out=w2_t, in_=w2_d[:, :, :])
                nc.sync.dma_start(out=ones_t, in_=ones_d[:, :])
                nc.sync.dma_start(out=b2_t, in_=b2_d[:, :])

            insts = [dict(idx=i) for i in range(N)]

            # ---------------- load + LN1 stats -----------------------------
            def ld(st):
                st["x_tm"] = xp.tile([128, TPG, C], F32, tag="xtm",
                                     name=f"xtm{st['idx']}")
                row0 = (st["idx"] % NG) * GTOK
                for it in range(TPG):
                    r = row0 + it * 128
                    nc.sync.dma_start(out=st["x_tm"][:, it, :],
                                      in_=x_in[r:r + 128, :])

            def stats(st, which):
                src = st["x_tm"] if which == 1 else st["x2_tm"]
                mvb = sp.tile([128, TPG, 2], F32, tag=f"mvb{which}",
                              name=f"mvb{which}{st['idx']}")
                for it in range(TPG):
                    st6 = sp.tile([128, 6], F32, tag=f"st6{which}", bufs=2)
                    nc.vector.bn_stats(out=st6, in_=src[:, it, :])
                    nc.vector.bn_aggr(out=mvb[:, it, :], in_=st6)
                st[f"mvb{which}"] = mvb

            def rsq(st, which):
                """Pool: r1 = rsqrt(var+eps), mnr = -mean*r1 (2 Newton).

                Pool only passes the ISA check for TensorTensor/TensorCopy/
                Memset, so every scalar rides a broadcast constant tile
                (cst_t[:, j] = [128, TPG, 1]): 0=eps 1=int(1) 2=magic
                3=-0.5 4=1.5 5=-1.
                """
                mvb = st[f"mvb{which}"]
                S = [128, TPG, 1]
                tg = f"rq{which}"
                TT = nc.gpsimd.tensor_tensor
                # eps-add + bit hack on DVE (Pool shifts need i64 out);
                # the Newton steps run on Pool.
                vpe = sp.tile(S, F32, tag=f"vpe{tg}")
                nc.vector.tensor_scalar(out=vpe, in0=mvb[:, :, 1:2],
                                        scalar1=EPS, scalar2=None, op0=OP.add)
                i1 = sp.tile(S, I32, tag=f"i1{tg}")
                nc.vector.tensor_scalar(out=i1, in0=vpe.bitcast(I32),
                                        scalar1=1, scalar2=None,
                                        op0=OP.logical_shift_right)
                i2 = sp.tile(S, I32, tag=f"i2{tg}")
                nc.vector.tensor_scalar(out=i2, in0=i1, scalar1=MAGIC,
                                        scalar2=-1, op0=OP.subtract,
                                        op1=OP.mult)
                r0 = i2.bitcast(F32)
                t = sp.tile(S, F32, tag=f"t{tg}")
                TT(out=t, in0=vpe, in1=r0, op=OP.mult)
                TT(out=t, in0=t, in1=r0, op=OP.mult)
                TT(out=t, in0=t, in1=cst_t[:, 3], op=OP.mult)
                TT(out=t, in0=t, in1=cst_t[:, 4], op=OP.add)
                r1 = sp.tile(S, F32, tag=f"r1{tg}",
                             name=f"r1{tg}{st['idx']}")
                TT(out=r1, in0=t, in1=r0, op=OP.mult)
                t2 = sp.tile(S, F32, tag=f"t2{tg}")
                TT(out=t2, in0=vpe, in1=r1, op=OP.mult)
                TT(out=t2, in0=t2, in1=r1, op=OP.mult)
                TT(out=t2, in0=t2, in1=cst_t[:, 3], op=OP.mult)
                TT(out=t2, in0=t2, in1=cst_t[:, 4], op=OP.add)
                rb = sp.tile(S, F32, tag=f"rb{tg}",
                             name=f"rb{tg}{st['idx']}")
                TT(out=rb, in0=t2, in1=r1, op=OP.mult)
                mm = sp.tile(S, F32, tag=f"mm{tg}")
                TT(out=mm, in0=mvb[:, :, 0:1], in1=rb, op=OP.mult)
                mnr = sp.tile(S, F32, tag=f"mnr{tg}",
                              name=f"mnr{tg}{st['idx']}")
                TT(out=mnr, in0=mm, in1=cst_t[:, 5], op=OP.mult)
                st[f"r{which}"] = rb
                st[f"mnr{which}"] = mnr

            # ---------------- LN apply + DMA transpose ---------------------
            def xn_alloc(st, which):
                xn = gp.tile([128, TPG, CPAD], FP8, tag=f"xn{which}",
                             name=f"xn{which}_{st['idx']}")
                nc.gpsimd.memset(xn[:, :, C:CPAD], 0.0)
                st[f"xn{which}"] = xn

            def apply_ln(st, which):
                src = st["x_tm"] if which == 1 else st["x2_tm"]
                xn = st[f"xn{which}"]
                r1, mnr = st[f"r{which}"], st[f"mnr{which}"]
                for it in range(TPG):
                    nc.scalar.activation(out=xn[:, it, 0:C],
                                         in_=src[:, it, :],
                                         func=AF.Identity,
                                         scale=r1[:, it, :],
                                         bias=mnr[:, it, :])

            def dmat_ln(st, which):
                # one whole-group xbar transpose: in [128, 1024] u16,
                # out chunks (it, c) -> [128, 8, 128]
                xn16 = st[f"xn{which}"].bitcast(U16)
                xcm = gp.tile([128, 2 * TPG, 128], U16, tag=f"xcm{which}",
                              name=f"xcm{which}_{st['idx']}")
                nc.scalar.dma_start_transpose(
                    xcm, xn16.rearrange("p i t -> p (i t)"))
                st[f"xcm{which}"] = xcm

            def _rhs(st, which):
                """[128, c, s, it, t] fp8 view of the (it,c)-chunked xcm."""
                x8 = st[f"xcm{which}"].bitcast(FP8)
                return x8.rearrange("p (i c) (t s) -> p c s i t", c=2, s=2)

            # ---------------- projections ---------------------------------
            def pjA(st, m):
                """PE psu/psg matmuls + ACT tanh for m-tile."""
                if m == 0:
                    st["ybuf"] = gp.tile([128, 6, GTOK], FP8, tag="ybuf",
                                         name=f"ybuf{st['idx']}")
                    st["thp"] = gp.tile([128, KT, GTOK], BF16, tag="thp",
                                        name=f"thp{st['idx']}")
                    st["m2b"] = gp.tile([128, KT, GTOK], BF16, tag="m2b",
                                        name=f"m2b{st['idx']}")
                msl = slice(m * 128, (m + 1) * 128)
                rhs = _rhs(st, 1)
                psu = ps.tile([128, GTOK], F32, tag="ps", name="psu")
                psg = ps.tile([128, GTOK], F32, tag="ps", name="psg")
                for j in range(2):
                    nc.tensor.matmul(psu, wu_t[:, 2 * j:2 * j + 2, msl],
                                     rhs[:, j], start=(j == 0),
                                     stop=(j == 1), perf_mode=DR)
                for j in range(2):
                    nc.tensor.matmul(psg, wg_t[:, 2 * j:2 * j + 2, msl],
                                     rhs[:, j], start=(j == 0),
                                     stop=(j == 1), perf_mode=DR)
                thraw = tp.tile([128, GTOK], BF16, tag="thraw", bufs=2,
                                name=f"thraw{st['idx']}_{m}")
                nc.scalar.activation(out=thraw, in_=psg, func=AF.Tanh,
                                     scale=SH, bias=bg_t[:, m:m + 1])
                st[f"thraw{m}"] = thraw
                st[f"psu{m}"] = psu

            def pjB(st, m):
                """DVE thp/thm/m2 + ACT ybuf copy for m-tile."""
                thraw, psu = st[f"thraw{m}"], st[f"psu{m}"]
                nc.vector.tensor_scalar(out=st["thp"][:, m, :], in0=thraw,
                                        scalar1=1.0, scalar2=SH,
                                        op0=OP.add, op1=OP.mult)
                thm = tp.tile([128, GTOK], BF16, tag="thm", bufs=2,
                              name=f"thm{st['idx']}_{m}")
                nc.vector.tensor_scalar(out=thm, in0=thraw,
                                        scalar1=-1.0, scalar2=SH,
                                        op0=OP.add, op1=OP.mult)
                nc.vector.scalar_tensor_tensor(
                    out=st["m2b"][:, m, :], in0=psu, scalar=bu_t[:, m:m + 1],
                    in1=thm, op0=OP.add, op1=OP.mult)
                nc.scalar.activation(out=st["ybuf"][:, KT + m, :],
                                     in_=st["m2b"][:, m, :], func=AF.Copy)

            # ---------------- scan (two independent half-token chains) ----
            HTOK = GTOK // 2

            def _hsl(hf):
                return slice(hf * HTOK, (hf + 1) * HTOK)

            def sp1(st, hf):
                p = scanp.tile([128, KT, HTOK], F32, tag="p",
                               name=f"scp{hf}")
                st[f"p{hf}"] = p
                y = st["ybuf"]
                for m in range(KT):
                    msl = slice(m * 128, (m + 1) * 128)
                    nc.tensor.matmul(p[:, m, :], a_t[:, 3:5, msl],
                                     y[:, 3:5, _hsl(hf)], start=True,
                                     stop=False, perf_mode=DR)
                    nc.tensor.matmul(p[:, m, :], a_t[:, 5, msl],
                                     y[:, 5, _hsl(hf)], start=False,
                                     stop=True)

            def sry(st, hf):
                nc.vector.tensor_mul(out=st["ybuf"][:, 0:KT, _hsl(hf)],
                                     in0=st[f"p{hf}"],
                                     in1=st["thp"][:, :, _hsl(hf)])

            def srmm(st, hf):
                p, y = st[f"p{hf}"], st["ybuf"]
                for m in range(KT):
                    msl = slice(m * 128, (m + 1) * 128)
                    for j in range(KT):
                        nc.tensor.matmul(p[:, m, :],
                                         a_t[:, 2 * j:2 * j + 2, msl],
                                         y[:, 2 * j:2 * j + 2, _hsl(hf)],
                                         start=(j == 0), stop=(j == KT - 1),
                                         perf_mode=DR)

            def sfy(st, hf):
                if hf == 0:
                    st["t1"] = tp.tile([128, KT, GTOK], BF16, tag="t1",
                                       bufs=2, name=f"t1f{st['idx']}")
                nc.vector.tensor_mul(out=st["t1"][:, :, _hsl(hf)],
                                     in0=st[f"p{hf}"],
                                     in1=st["thp"][:, :, _hsl(hf)])

            def sfh(st, hf):
                # h stored half-major [p, hf, c, t] so the xbar DMA input
                # (c t) flattens contiguously.
                if hf == 0:
                    st["h_cm"] = gp.tile([128, 2, KT, HTOK], BF16, tag="hcm",
                                         name=f"hcm{st['idx']}")
                nc.vector.tensor_add(out=st["h_cm"][:, hf],
                                     in0=st["t1"][:, :, _hsl(hf)],
                                     in1=st["m2b"][:, :, _hsl(hf)])

            # ---------------- residual 1 (DMA transpose + Pool) -----------
            def dthh(st, hf):
                # h_tm layout [p, hf, c, it2, q]: one xbar DMA per half
                # ([128, 3*256] -> [128, 6, 128], chunks (c, it2) contiguous).
                if hf == 0:
                    st["h_tm"] = gp.tile([128, 2, KT, 2, 128], BF16,
                                         tag="htm", name=f"htm{st['idx']}")
                nc.scalar.dma_start_transpose(
                    st["h_tm"][:, hf].rearrange("p c i q -> p (c i) q"),
                    st["h_cm"][:, hf].rearrange("p c t -> p (c t)"))

            def rs(st, hf):
                if hf == 0:
                    st["x2_tm"] = x2p.tile([128, TPG, C], F32, tag="x2tm",
                                           name=f"x2tm{st['idx']}")
                isl = slice(2 * hf, 2 * hf + 2)
                x2r = st["x2_tm"][:, isl, :].rearrange(
                    "p i (c q) -> p i c q", c=KT)
                xr = st["x_tm"][:, isl, :].rearrange(
                    "p i (c q) -> p i c q", c=KT)
                htm = st["h_tm"][:, hf].rearrange("p c i q -> p i c q")
                nc.gpsimd.tensor_sub(out=x2r, in0=xr, in1=htm)

            # ---------------- MLP -----------------------------------------
            def mh(st, i):
                if i == 0:
                    st["hid"] = gp.tile([128, MH, GTOK], FP8, tag="hid",
                                        name=f"hid{st['idx']}")
                hsl = slice(i * 128, (i + 1) * 128)
                rhs = _rhs(st, 2)
                psh = ps.tile([128, GTOK], F32, tag="ps", name="psh")
                for j in range(2):
                    nc.tensor.matmul(psh, w1_t[:, 2 * j:2 * j + 2, hsl],
                                     rhs[:, j], start=(j == 0),
                                     stop=(j == 1), perf_mode=DR)
                nc.scalar.activation(out=st["hid"][:, i, :], in_=psh,
                                     func=AF.Gelu_apprx_tanh, scale=IWS,
                                     bias=b1_t[:, i:i + 1])

            def mo(st, it):
                if it == 0:
                    st["ob"] = x2p.tile([128, TPG, C], F32, tag="ob",
                                        name=f"ob{st['idx']}")
                tsl = slice(it * 128, (it + 1) * 128)
                pso = ps.tile([128, C], F32, tag="pso", name="pso", bufs=1)
                for j in range(MH // 2):
                    nc.tensor.matmul(pso,
                                     st["hid"][:, 2 * j:2 * j + 2, tsl],
                                     w2_t[:, 2 * j:2 * j + 2, :],
                                     start=(j == 0), stop=False,
                                     perf_mode=DR)
                nc.tensor.matmul(pso, ones_t, b2_t, start=False, stop=True)
                nc.vector.scalar_tensor_tensor(
                    out=st["ob"][:, it, :], in0=pso, scalar=IWS,
                    in1=st["x2_tm"][:, it, :], op0=OP.mult, op1=OP.add)
                row0 = (st["idx"] % NG) * GTOK + it * 128
                nc.scalar.dma_start(out=out_d[row0:row0 + 128, :],
                                    in_=st["ob"][:, it, :])

            # ---------------- conductor -----------------------------------
            ld(insts[0])
            load_late_weights()
            xn_alloc(insts[0], 1)
            stats(insts[0], 1)
            rsq(insts[0], 1)
            apply_ln(insts[0], 1)
            dmat_ln(insts[0], 1)
            if N > 1:
                ld(insts[1])
                xn_alloc(insts[1], 1)
            for m in range(KT):
                pjA(insts[0], m)
                pjB(insts[0], m)
            if N > 1:
                stats(insts[1], 1)
                rsq(insts[1], 1)
                apply_ln(insts[1], 1)
                dmat_ln(insts[1], 1)

            def rnd(S):
                for hf in range(2):
                    sry(S, hf)
                    srmm(S, hf)

            for k in range(N + 1):
                S = insts[k] if k < N else None
                P = insts[k + 1] if k + 1 < N else None
                Q = insts[k + 2] if k + 2 < N else None
                M = insts[k - 1] if 1 <= k else None

                if Q:
                    ld(Q)
                if M:
                    rs(M, 0)
                    rs(M, 1)
                    xn_alloc(M, 2)
                if Q:
                    xn_alloc(Q, 1)
                if P:
                    pjA(P, 0)
                if S:
                    sp1(S, 0)
                    sp1(S, 1)
                    rnd(S)          # round 1
                if P:
                    pjB(P, 0)
                if S:
                    rnd(S)          # round 2
                if Q:
                    stats(Q, 1)
                    rsq(Q, 1)
                if P:
                    pjA(P, 1)
                if S:
                    rnd(S)          # round 3
                if M:
                    stats(M, 2)
                    rsq(M, 2)
                if P:
                    pjA(P, 2)
                if S:
                    rnd(S)          # round 4
                if P:
                    pjB(P, 1)
                if Q:
                    apply_ln(Q, 1)
                    dmat_ln(Q, 1)
                if M:
                    apply_ln(M, 2)
                    dmat_ln(M, 2)
                if S:
                    rnd(S)          # round 5
                if P:
                    pjB(P, 2)
                if M:
                    for i in range(MH):
                        mh(M, i)
                if S:
                    # round 6 fused with the scan tail, half by half
                    sry(S, 0)
                    srmm(S, 0)
                    sfy(S, 0)
                    sfh(S, 0)
                    dthh(S, 0)
                    sry(S, 1)
                    srmm(S, 1)
                    sfy(S, 1)
                    sfh(S, 1)
                    dthh(S, 1)
                if M:
                    mo(M, 0)
                    mo(M, 1)
                    mo(M, 2)
                    mo(M, 3)
    return nc


_NC_CACHE = {}


def _get_nc():
    if "nc" not in _NC_CACHE:
        _NC_CACHE["nc"] = _patch_nc(build_nc())
    return _NC_CACHE["nc"]


def _perm_rows(w, bias_unused=None):
    """[C, D] -> [128, 4, D]: block (c,s) partition p = row 256c+2p+s."""
    D = w.shape[1]
    out = np.zeros((128, 4, D), np.float32)
    for c in range(2):
        for s in range(2):
            rows = 256 * c + 2 * np.arange(128) + s
            valid = rows < C
            out[valid, 2 * c + s, :] = w[rows[valid], :]
    return out


# ---------------------------------------------------------------- kernel --
def kernel(x, norm1_scale, norm1_bias, Wu, bu, Wg, bg, A,
           norm2_scale, norm2_bias, mlp_w1, mlp_b1, mlp_w2, mlp_b2,
           _return_raw=False):
    import ml_dtypes
    E4 = ml_dtypes.float8_e4m3
    f = np.float32
    x = np.asarray(x, f)
    norm1_scale = np.asarray(norm1_scale, f)
    norm1_bias = np.asarray(norm1_bias, f)
    Wu, bu = np.asarray(Wu, f), np.asarray(bu, f)
    Wg, bg = np.asarray(Wg, f), np.asarray(bg, f)
    A = np.asarray(A, f)
    norm2_scale = np.asarray(norm2_scale, f)
    norm2_bias = np.asarray(norm2_bias, f)
    mlp_w1, mlp_b1 = np.asarray(mlp_w1, f), np.asarray(mlp_b1, f)
    mlp_w2, mlp_b2 = np.asarray(mlp_w2, f), np.asarray(mlp_b2, f)

    # fold LN affine into downstream weights
    wu = norm1_scale[:, None] * Wu
    bu_f = bu + norm1_bias @ Wu
    wg = norm1_scale[:, None] * Wg
    bg_f = bg + norm1_bias @ Wg
    w1 = norm2_scale[:, None] * mlp_w1
    b1_f = mlp_b1 + norm2_bias @ mlp_w1

    wu_p = (_perm_rows(wu) * WS).astype(E4)
    wg_p = (_perm_rows(wg) * WS).astype(E4)
    w1_p = (_perm_rows(w1) * WS).astype(E4)

    a_p = np.zeros((128, 6, C), f)
    for s in range(6):
        a_p[:, s, :] = A[(s % 3) * 128:(s % 3) * 128 + 128, :]
    a_p = (a_p * WS).astype(E4)

    w2_p = np.zeros((128, MH, C), f)
    for s in range(MH):
        w2_p[:, s, :] = mlp_w2[s * 128:(s + 1) * 128, :]
    w2_p = (w2_p * WS).astype(E4)

    b2_p = (mlp_b2[None, :] * WS).astype(E4)
    ones_p = np.ones((1, 128), f).astype(E4)
    bu_p = np.ascontiguousarray((bu_f * WS).reshape(KT, 128).T)
    bg_p = np.ascontiguousarray((bg_f * 0.5).reshape(KT, 128).T)
    b1_p = np.ascontiguousarray(b1_f.reshape(MH, 128).T)

    cvals = np.array([EPS,
                      np.int32(1).view(f),
                      np.int32(MAGIC).view(f),
                      -0.5, 1.5, -1.0], f)
    cst_p = np.broadcast_to(cvals[None, :, None, None],
                            (128, 6, TPG, 1)).astype(f).copy()

    xs = x.reshape(NCORES, NTOK, C)
    in_maps = [{
        "x": np.ascontiguousarray(xs[i]),
        "wu": wu_p, "wg": wg_p, "a": a_p, "w1": w1_p, "w2": w2_p,
        "ones": ones_p, "b2": b2_p, "bu": bu_p, "bg": bg_p, "b1": b1_p,
        "cst": cst_p,
    } for i in range(NCORES)]

    res = run_bass_kernel_spmd(_get_nc(), in_maps, list(range(NCORES)))
    if _return_raw:
        return res
    out = np.concatenate([res.results[i]["out"] for i in range(NCORES)],
                         axis=0)
    return out.reshape(B, H, W, C).astype(np.float32)


# revision 35
# speedup vs baseline: 2.4014x; 1.4590x over previous
"""CSSM TinyViT block on 8 TRN2 NeuronCores — DMA-xbar-transpose version.

Strategy
--------
Data-parallel over batch: B=16 -> 2 samples (2048 tokens) per core, processed
as 4 groups of 512 tokens in a software pipeline.

All channel-mixing matmuls run fp8(e4m3) DoubleRow.  Token-major -> channel-
major layout changes go through the DMA crossbar transpose (u16 views of fp8
pairs; the channel order becomes (256c + 2p + s), which is folded into the
weight row order host-side).  h comes back token-major the same way in bf16.
This removes every PE transpose and every PSUM->SBUF copy of the baseline.

Biases ride the epilogues: bg via the tanh ACT bias AP, bu via the m2 STT
scalar AP, b1 via the gelu ACT bias AP, b2 via the ones-row matmul.

Engine placement: DVE keeps only PSUM-coupled elementwise work (scan gate
multiplies, m2, m_out epilogue, bn_stats); the LN rsqrt chains (rsqrt bit
hack + 1 Newton step), the residual subtract and the xn pad memsets run on
Pool; LN applies, tanh/gelu and the m2->fp8 copies run on ACT.

The emission order interleaves PE-decoupled DVE filler (proj epilogues,
bn_stats of neighbour groups, m_out) between the scan rounds so the DVE never
waits on the scan matmuls.

PSUM: scan p [128,3,512] f32 (3 banks) + 4-deep 1-bank ring (proj/MLP) = 7.
"""
import json
import os
import types

import numpy as np

import concourse.bass as bass
import concourse.mybir as mybir
from concourse.tile import TileContext
from concourse.bass_utils import run_bass_kernel_spmd

F32 = mybir.dt.float32
FP8 = mybir.dt.float8e4
BF16 = mybir.dt.bfloat16
U16 = mybir.dt.uint16
I32 = mybir.dt.int32
AF = mybir.ActivationFunctionType
OP = mybir.AluOpType
DR = mybir.MatmulPerfMode.DoubleRow

B, H, W, C, T = 16, 32, 32, 384, 8
HID = 4 * C
EPS = 1e-6
NCORES = 8
BSH = B // NCORES
NTOK = BSH * H * W             # 2048 tokens per core
GTOK = 512                     # tokens per group
NG = NTOK // GTOK              # 4 groups
TPG = GTOK // 128              # 4 token-tiles per group
KT = C // 128                  # 3 channel tiles
MH = HID // 128                # 12 hidden tiles
WS = 64.0                      # fp8 weight scale
SH = 0.5 / WS                  # dequant incl. the tanh 0.5
IWS = 1.0 / WS
MAGIC = 0x5F3759DF
CPAD = 512                     # xn padded channel count (u16-transposable)


# ---------------------------------------------------------------- bir fix --
# This container's walrus rejects instructions whose sync-wait list exceeds
# the opcode's wait slots.  Move excess waits onto EventSemaphore
# instructions inserted before the instruction on the same engine queue.
_WAIT_LIMITS = {"Drain": 0, "DmaTransposeAnt": 0}
_WAIT_DEFAULT = 1


def _fix_bir_json(bj: bytes) -> bytes:
    bir = json.loads(bj)
    counter = [0]

    def fix_blocks(blocks):
        for b in blocks:
            insts = b.get("instructions")
            if insts:
                new = []
                for inst in insts:
                    si = inst.get("sync_info")
                    waits = (si or {}).get("on_wait") or []
                    limit = _WAIT_LIMITS.get(inst.get("opcode"), _WAIT_DEFAULT)
                    if len(waits) > limit:
                        n_extra = len(waits) - limit
                        extra, keep = waits[:n_extra], waits[n_extra:]
                        for wv in extra:
                            counter[0] += 1
                            new.append({
                                "name": f"I-wfix-{counter[0]}",
                                "opcode": "EventSemaphore",
                                "engine": inst["engine"],
                                "ins": [],
                                "outs": [],
                                "sync_info": {"on_update": [], "on_wait": [wv]},
                                "debug": inst.get("debug", 0),
                            })
                        si["on_wait"] = keep
                    new.append(inst)
                b["instructions"] = new
            fix_blocks(b.get("blocks") or [])

    for fn in bir.get("functions", []):
        fix_blocks(fn.get("blocks") or [])
    return json.dumps(bir).encode()


def _patch_nc(nc):
    orig = nc.to_json_bytes

    def to_json_bytes(self):
        return _fix_bir_json(orig())

    nc.to_json_bytes = types.MethodType(to_json_bytes, nc)
    return nc


# ----------------------------------------------------------- device build --
def build_nc(repeat=1):
    nc = bass.Bass()

    x_in = nc.declare_dram_parameter("x", [NTOK, C], F32, isOutput=False)
    wu_d = nc.declare_dram_parameter("wu", [128, 4, C], FP8, isOutput=False)
    wg_d = nc.declare_dram_parameter("wg", [128, 4, C], FP8, isOutput=False)
    a_d = nc.declare_dram_parameter("a", [128, 6, C], FP8, isOutput=False)
    w1_d = nc.declare_dram_parameter("w1", [128, 4, HID], FP8, isOutput=False)
    w2_d = nc.declare_dram_parameter("w2", [128, MH, C], FP8, isOutput=False)
    ones_d = nc.declare_dram_parameter("ones", [1, 128], FP8, isOutput=False)
    b2_d = nc.declare_dram_parameter("b2", [1, C], FP8, isOutput=False)
    bu_d = nc.declare_dram_parameter("bu", [128, KT], F32, isOutput=False)
    bg_d = nc.declare_dram_parameter("bg", [128, KT], F32, isOutput=False)
    b1_d = nc.declare_dram_parameter("b1", [128, MH], F32, isOutput=False)
    cst_d = nc.declare_dram_parameter("cst", [128, 9, TPG, 1], F32,
                                      isOutput=False)
    out_d = nc.declare_dram_parameter("out", [NTOK, C], F32, isOutput=True)

    N = NG * repeat

    with TileContext(nc) as tc:
        with (
            tc.tile_pool(name="wp", bufs=1) as wp,
            tc.tile_pool(name="xp", bufs=4) as xp,
            tc.tile_pool(name="x2p", bufs=2) as x2p,
            tc.tile_pool(name="gp", bufs=2) as gp,
            tc.tile_pool(name="tp", bufs=2) as tp,
            tc.tile_pool(name="sp", bufs=2) as sp,
            tc.tile_pool(name="ps", bufs=3, space="PSUM") as ps,
            tc.tile_pool(name="scanp", bufs=2, space="PSUM") as scanp,
        ):
            # ---- weights / constants ----
            wu_t = wp.tile([128, 4, C], FP8, tag="wu")
            wg_t = wp.tile([128, 4, C], FP8, tag="wg")
            a_t = wp.tile([128, 6, C], FP8, tag="a")
            w1_t = wp.tile([128, 4, HID], FP8, tag="w1")
            w2_t = wp.tile([128, MH, C], FP8, tag="w2")
            ones_t = wp.tile([1, 128], FP8, tag="ones")
            b2_t = wp.tile([1, C], FP8, tag="b2")
            bu_t = wp.tile([128, KT], F32, tag="bu")
            bg_t = wp.tile([128, KT], F32, tag="bg")
            b1_t = wp.tile([128, MH], F32, tag="b1")
            cst_t = wp.tile([128, 9, TPG, 1], F32, tag="cst")
            nc.sync.dma_start(out=cst_t, in_=cst_d[:, :, :, :])
            nc.sync.dma_start(out=wu_t, in_=wu_d[:, :, :])
            nc.sync.dma_start(out=wg_t, in_=wg_d[:, :, :])
            nc.sync.dma_start(out=a_t, in_=a_d[:, :, :])
            nc.sync.dma_start(out=bu_t, in_=bu_d[:, :])
            nc.sync.dma_start(out=bg_t, in_=bg_d[:, :])
            nc.sync.dma_start(out=b1_t, in_=b1_d[:, :])

            def load_late_weights():
                nc.sync.dma_start(out=w1_t, in_=w1_d[:, :, :])
                nc.sync.dma_start(out=w2_t, in_=w2_d[:, :, :])
                nc.sync.dma_start(out=ones_t, in_=ones_d[:, :])
                nc.sync.dma_start(out=b2_t, in_=b2_d[:, :])

            insts = [dict(idx=i) for i in range(N)]

            # ---------------- load + LN1 stats -----------------------------
            def ld(st):
                st["x_tm"] = xp.tile([128, TPG, C], F32, tag="xtm",
                                     name=f"xtm{st['idx']}")
                row0 = (st["idx"] % NG) * GTOK
                for it in range(TPG):
                    r = row0 + it * 128
                    nc.sync.dma_start(out=st["x_tm"][:, it, :],
                                      in_=x_in[r:r + 128, :])

            def stats(st, which):
                """DVE bn_stats per tile; the even/odd combine runs on Pool.

                mean = (me+mo)/2; var = (nve+nvo)/C + ((me-mo)/2)^2 * ...
                with n_e = n_o = C/2:  var = (nve+nvo)/C + (me-mo)^2/4.
                cst_t[:, 6]=1/C  cst_t[:, 7]=0.25
                """
                src = st["x_tm"] if which == 1 else st["x2_tm"]
                st6 = sp.tile([128, TPG, 6], F32, tag=f"st6{which}",
                              name=f"st6_{which}_{st['idx']}")
                for it in range(TPG):
                    nc.vector.bn_stats(out=st6[:, it, :], in_=src[:, it, :])
                TT = nc.gpsimd.tensor_tensor
                S = [128, TPG, 1]
                tg = f"ag{which}"
                me, mo = st6[:, :, 1:2], st6[:, :, 4:5]
                nve, nvo = st6[:, :, 2:3], st6[:, :, 5:6]
                mvb = sp.tile([128, TPG, 2], F32, tag=f"mvb{which}",
                              name=f"mvb{which}{st['idx']}")
                ms = sp.tile(S, F32, tag=f"ms{tg}")
                TT(out=ms, in0=me, in1=mo, op=OP.add)
                TT(out=mvb[:, :, 0:1], in0=ms, in1=cst_t[:, 8], op=OP.mult)
                d = sp.tile(S, F32, tag=f"d{tg}")
                TT(out=d, in0=me, in1=mo, op=OP.subtract)
                d2 = sp.tile(S, F32, tag=f"d2{tg}")
                TT(out=d2, in0=d, in1=d, op=OP.mult)
                dq = sp.tile(S, F32, tag=f"dq{tg}")
                TT(out=dq, in0=d2, in1=cst_t[:, 7], op=OP.mult)
                vs = sp.tile(S, F32, tag=f"vs{tg}")
                TT(out=vs, in0=nve, in1=nvo, op=OP.add)
                vv = sp.tile(S, F32, tag=f"vv{tg}")
                TT(out=vv, in0=vs, in1=cst_t[:, 6], op=OP.mult)
                TT(out=mvb[:, :, 1:2], in0=vv, in1=dq, op=OP.add)
                st[f"mvb{which}"] = mvb

            def rsq(st, which):
                """Pool: r1 = rsqrt(var+eps), mnr = -mean*r1 (2 Newton).

                Pool only passes the ISA check for TensorTensor/TensorCopy/
                Memset, so every scalar rides a broadcast constant tile
                (cst_t[:, j] = [128, TPG, 1]): 0=eps 1=int(1) 2=magic
                3=-0.5 4=1.5 5=-1.
                """
                mvb = st[f"mvb{which}"]
                S = [128, TPG, 1]
                tg = f"rq{which}"
                TT = nc.gpsimd.tensor_tensor
                # eps-add + bit hack on DVE (Pool shifts need i64 out);
                # the Newton steps run on Pool.
                vpe = sp.tile(S, F32, tag=f"vpe{tg}")
                nc.vector.tensor_scalar(out=vpe, in0=mvb[:, :, 1:2],
                                        scalar1=EPS, scalar2=None, op0=OP.add)
                i1 = sp.tile(S, I32, tag=f"i1{tg}")
                nc.vector.tensor_scalar(out=i1, in0=vpe.bitcast(I32),
                                        scalar1=1, scalar2=None,
                                        op0=OP.logical_shift_right)
                i2 = sp.tile(S, I32, tag=f"i2{tg}")
                nc.vector.tensor_scalar(out=i2, in0=i1, scalar1=MAGIC,
                                        scalar2=-1, op0=OP.subtract,
                                        op1=OP.mult)
                r0 = i2.bitcast(F32)
                t = sp.tile(S, F32, tag=f"t{tg}")
                TT(out=t, in0=vpe, in1=r0, op=OP.mult)
                TT(out=t, in0=t, in1=r0, op=OP.mult)
                TT(out=t, in0=t, in1=cst_t[:, 3], op=OP.mult)
                TT(out=t, in0=t, in1=cst_t[:, 4], op=OP.add)
                r1 = sp.tile(S, F32, tag=f"r1{tg}",
                             name=f"r1{tg}{st['idx']}")
                TT(out=r1, in0=t, in1=r0, op=OP.mult)
                t2 = sp.tile(S, F32, tag=f"t2{tg}")
                TT(out=t2, in0=vpe, in1=r1, op=OP.mult)
                TT(out=t2, in0=t2, in1=r1, op=OP.mult)
                TT(out=t2, in0=t2, in1=cst_t[:, 3], op=OP.mult)
                TT(out=t2, in0=t2, in1=cst_t[:, 4], op=OP.add)
                rb = sp.tile(S, F32, tag=f"rb{tg}",
                             name=f"rb{tg}{st['idx']}")
                TT(out=rb, in0=t2, in1=r1, op=OP.mult)
                mm = sp.tile(S, F32, tag=f"mm{tg}")
                TT(out=mm, in0=mvb[:, :, 0:1], in1=rb, op=OP.mult)
                mnr = sp.tile(S, F32, tag=f"mnr{tg}",
                              name=f"mnr{tg}{st['idx']}")
                TT(out=mnr, in0=mm, in1=cst_t[:, 5], op=OP.mult)
                st[f"r{which}"] = rb
                st[f"mnr{which}"] = mnr

            # ---------------- LN apply + DMA transpose ---------------------
            def xn_alloc(st, which):
                xn = gp.tile([128, TPG, CPAD], FP8, tag=f"xn{which}",
                             name=f"xn{which}_{st['idx']}")
                nc.gpsimd.memset(xn[:, :, C:CPAD], 0.0)
                st[f"xn{which}"] = xn

            def apply_ln(st, which):
                src = st["x_tm"] if which == 1 else st["x2_tm"]
                xn = st[f"xn{which}"]
                r1, mnr = st[f"r{which}"], st[f"mnr{which}"]
                for it in range(TPG):
                    nc.scalar.activation(out=xn[:, it, 0:C],
                                         in_=src[:, it, :],
                                         func=AF.Identity,
                                         scale=r1[:, it, :],
                                         bias=mnr[:, it, :])

            def dmat_ln(st, which):
                # one whole-group xbar transpose: in [128, 1024] u16,
                # out chunks (it, c) -> [128, 8, 128]
                xn16 = st[f"xn{which}"].bitcast(U16)
                xcm = gp.tile([128, 2 * TPG, 128], U16, tag=f"xcm{which}",
                              name=f"xcm{which}_{st['idx']}")
                nc.scalar.dma_start_transpose(
                    xcm, xn16.rearrange("p i t -> p (i t)"))
                st[f"xcm{which}"] = xcm

            def _rhs(st, which):
                """[128, c, s, it, t] fp8 view of the (it,c)-chunked xcm."""
                x8 = st[f"xcm{which}"].bitcast(FP8)
                return x8.rearrange("p (i c) (t s) -> p c s i t", c=2, s=2)

            # ---------------- projections ---------------------------------
            def pjA(st, m):
                """PE psu/psg matmuls + ACT tanh for m-tile."""
                if m == 0:
                    st["ybuf"] = gp.tile([128, 6, GTOK], FP8, tag="ybuf",
                                         name=f"ybuf{st['idx']}")
                    st["thp"] = gp.tile([128, KT, GTOK], BF16, tag="thp",
                                        name=f"thp{st['idx']}")
                    st["m2b"] = gp.tile([128, KT, GTOK], BF16, tag="m2b",
                                        name=f"m2b{st['idx']}")
                msl = slice(m * 128, (m + 1) * 128)
                rhs = _rhs(st, 1)
                psu = ps.tile([128, GTOK], F32, tag="ps", name="psu")
                psg = ps.tile([128, GTOK], F32, tag="ps", name="psg")
                for j in range(2):
                    nc.tensor.matmul(psu, wu_t[:, 2 * j:2 * j + 2, msl],
                                     rhs[:, j], start=(j == 0),
                                     stop=(j == 1), perf_mode=DR)
                for j in range(2):
                    nc.tensor.matmul(psg, wg_t[:, 2 * j:2 * j + 2, msl],
                                     rhs[:, j], start=(j == 0),
                                     stop=(j == 1), perf_mode=DR)
                thraw = tp.tile([128, GTOK], BF16, tag="thraw", bufs=2,
                                name=f"thraw{st['idx']}_{m}")
                nc.scalar.activation(out=thraw, in_=psg, func=AF.Tanh,
                                     scale=SH, bias=bg_t[:, m:m + 1])
                ub = tp.tile([128, GTOK], BF16, tag="ub", bufs=2,
                             name=f"ub{st['idx']}_{m}")
                nc.scalar.activation(out=ub, in_=psu, func=AF.Identity,
                                     scale=1.0, bias=bu_t[:, m:m + 1])
                st[f"thraw{m}"] = thraw
                st[f"ub{m}"] = ub

            def pjB(st, m):
                """DVE thp/thm + Pool m2 + ACT ybuf copy for m-tile."""
                thraw, ub = st[f"thraw{m}"], st[f"ub{m}"]
                nc.vector.tensor_scalar(out=st["thp"][:, m, :], in0=thraw,
                                        scalar1=1.0, scalar2=SH,
                                        op0=OP.add, op1=OP.mult)
                thm = tp.tile([128, GTOK], BF16, tag="thm", bufs=2,
                              name=f"thm{st['idx']}_{m}")
                nc.vector.tensor_scalar(out=thm, in0=thraw,
                                        scalar1=-1.0, scalar2=SH,
                                        op0=OP.add, op1=OP.mult)
                nc.gpsimd.tensor_mul(out=st["m2b"][:, m, :], in0=ub,
                                     in1=thm)
                nc.scalar.activation(out=st["ybuf"][:, KT + m, :],
                                     in_=st["m2b"][:, m, :], func=AF.Copy)

            # ---------------- scan (two independent half-token chains) ----
            HTOK = GTOK // 2

            def _hsl(hf):
                return slice(hf * HTOK, (hf + 1) * HTOK)

            def sp1(st, hf):
                p = scanp.tile([128, KT, HTOK], F32, tag="p",
                               name=f"scp{hf}")
                st[f"p{hf}"] = p
                y = st["ybuf"]
                for m in range(KT):
                    msl = slice(m * 128, (m + 1) * 128)
                    nc.tensor.matmul(p[:, m, :], a_t[:, 3:5, msl],
                                     y[:, 3:5, _hsl(hf)], start=True,
                                     stop=False, perf_mode=DR)
                    nc.tensor.matmul(p[:, m, :], a_t[:, 5, msl],
                                     y[:, 5, _hsl(hf)], start=False,
                                     stop=True)

            def sry(st, hf):
                nc.vector.tensor_mul(out=st["ybuf"][:, 0:KT, _hsl(hf)],
                                     in0=st[f"p{hf}"],
                                     in1=st["thp"][:, :, _hsl(hf)])

            def srmm(st, hf):
                p, y = st[f"p{hf}"], st["ybuf"]
                for m in range(KT):
                    msl = slice(m * 128, (m + 1) * 128)
                    for j in range(KT):
                        nc.tensor.matmul(p[:, m, :],
                                         a_t[:, 2 * j:2 * j + 2, msl],
                                         y[:, 2 * j:2 * j + 2, _hsl(hf)],
                                         start=(j == 0), stop=(j == KT - 1),
                                         perf_mode=DR)

            def sfy(st, hf):
                if hf == 0:
                    st["t1"] = tp.tile([128, KT, GTOK], BF16, tag="t1",
                                       bufs=2, name=f"t1f{st['idx']}")
                nc.vector.tensor_mul(out=st["t1"][:, :, _hsl(hf)],
                                     in0=st[f"p{hf}"],
                                     in1=st["thp"][:, :, _hsl(hf)])

            def sfh(st, hf):
                # h stored half-major [p, hf, c, t] so the xbar DMA input
                # (c t) flattens contiguously.
                if hf == 0:
                    st["h_cm"] = gp.tile([128, 2, KT, HTOK], BF16, tag="hcm",
                                         name=f"hcm{st['idx']}")
                nc.vector.tensor_add(out=st["h_cm"][:, hf],
                                     in0=st["t1"][:, :, _hsl(hf)],
                                     in1=st["m2b"][:, :, _hsl(hf)])

            # ---------------- residual 1 (DMA transpose + Pool) -----------
            def dthh(st, hf):
                # h_tm layout [p, hf, c, it2, q]: one xbar DMA per half
                # ([128, 3*256] -> [128, 6, 128], chunks (c, it2) contiguous).
                if hf == 0:
                    st["h_tm"] = gp.tile([128, 2, KT, 2, 128], BF16,
                                         tag="htm", name=f"htm{st['idx']}")
                nc.scalar.dma_start_transpose(
                    st["h_tm"][:, hf].rearrange("p c i q -> p (c i) q"),
                    st["h_cm"][:, hf].rearrange("p c t -> p (c t)"))

            def rs(st, hf):
                if hf == 0:
                    st["x2_tm"] = x2p.tile([128, TPG, C], F32, tag="x2tm",
                                           name=f"x2tm{st['idx']}")
                isl = slice(2 * hf, 2 * hf + 2)
                x2r = st["x2_tm"][:, isl, :].rearrange(
                    "p i (c q) -> p i c q", c=KT)
                xr = st["x_tm"][:, isl, :].rearrange(
                    "p i (c q) -> p i c q", c=KT)
                htm = st["h_tm"][:, hf].rearrange("p c i q -> p i c q")
                nc.gpsimd.tensor_sub(out=x2r, in0=xr, in1=htm)

            # ---------------- MLP -----------------------------------------
            def mh(st, i):
                if i == 0:
                    st["hid"] = gp.tile([128, MH, GTOK], FP8, tag="hid",
                                        name=f"hid{st['idx']}")
                hsl = slice(i * 128, (i + 1) * 128)
                rhs = _rhs(st, 2)
                psh = ps.tile([128, GTOK], F32, tag="ps", name="psh")
                for j in range(2):
                    nc.tensor.matmul(psh, w1_t[:, 2 * j:2 * j + 2, hsl],
                                     rhs[:, j], start=(j == 0),
                                     stop=(j == 1), perf_mode=DR)
                nc.scalar.activation(out=st["hid"][:, i, :], in_=psh,
                                     func=AF.Gelu_apprx_tanh, scale=IWS,
                                     bias=b1_t[:, i:i + 1])

            def mo(st, it):
                if it == 0:
                    st["ob"] = x2p.tile([128, TPG, C], F32, tag="ob",
                                        name=f"ob{st['idx']}")
                tsl = slice(it * 128, (it + 1) * 128)
                pso = ps.tile([128, C], F32, tag="pso", name="pso", bufs=1)
                for j in range(MH // 2):
                    nc.tensor.matmul(pso,
                                     st["hid"][:, 2 * j:2 * j + 2, tsl],
                                     w2_t[:, 2 * j:2 * j + 2, :],
                                     start=(j == 0), stop=False,
                                     perf_mode=DR)
                nc.tensor.matmul(pso, ones_t, b2_t, start=False, stop=True)
                nc.vector.scalar_tensor_tensor(
                    out=st["ob"][:, it, :], in0=pso, scalar=IWS,
                    in1=st["x2_tm"][:, it, :], op0=OP.mult, op1=OP.add)
                row0 = (st["idx"] % NG) * GTOK + it * 128
                nc.scalar.dma_start(out=out_d[row0:row0 + 128, :],
                                    in_=st["ob"][:, it, :])

            # ---------------- conductor -----------------------------------
            ld(insts[0])
            load_late_weights()
            xn_alloc(insts[0], 1)
            stats(insts[0], 1)
            rsq(insts[0], 1)
            apply_ln(insts[0], 1)
            dmat_ln(insts[0], 1)
            if N > 1:
                ld(insts[1])
                xn_alloc(insts[1], 1)
            for m in range(KT):
                pjA(insts[0], m)
                pjB(insts[0], m)
            if N > 1:
                stats(insts[1], 1)
                rsq(insts[1], 1)
                apply_ln(insts[1], 1)
                dmat_ln(insts[1], 1)

            def rnd(S):
                for hf in range(2):
                    sry(S, hf)
                    srmm(S, hf)

            for k in range(N + 1):
                S = insts[k] if k < N else None
                P = insts[k + 1] if k + 1 < N else None
                Q = insts[k + 2] if k + 2 < N else None
                M = insts[k - 1] if 1 <= k else None

                if Q:
                    ld(Q)
                if M:
                    rs(M, 0)
                    rs(M, 1)
                    xn_alloc(M, 2)
                if Q:
                    xn_alloc(Q, 1)
                if P:
                    pjA(P, 0)
                if S:
                    sp1(S, 0)
                    sp1(S, 1)
                    rnd(S)          # round 1
                if P:
                    pjB(P, 0)
                if S:
                    rnd(S)          # round 2
                if Q:
                    stats(Q, 1)
                    rsq(Q, 1)
                if P:
                    pjA(P, 1)
                if S:
                    rnd(S)          # round 3
                if M:
                    stats(M, 2)
                    rsq(M, 2)
                if P:
                    pjA(P, 2)
                if S:
                    rnd(S)          # round 4
                if P:
                    pjB(P, 1)
                if Q:
                    apply_ln(Q, 1)
                    dmat_ln(Q, 1)
                if M:
                    apply_ln(M, 2)
                    dmat_ln(M, 2)
                if S:
                    rnd(S)          # round 5
                if P:
                    pjB(P, 2)
                if M:
                    for i in range(MH):
                        mh(M, i)
                if S:
                    # round 6 fused with the scan tail, half by half
                    sry(S, 0)
                    srmm(S, 0)
                    sfy(S, 0)
                    sfh(S, 0)
                    dthh(S, 0)
                    sry(S, 1)
                    srmm(S, 1)
                    sfy(S, 1)
                    sfh(S, 1)
                    dthh(S, 1)
                if M:
                    mo(M, 0)
                    mo(M, 1)
                    mo(M, 2)
                    mo(M, 3)
    return nc


_NC_CACHE = {}


def _get_nc():
    if "nc" not in _NC_CACHE:
        _NC_CACHE["nc"] = _patch_nc(build_nc())
    return _NC_CACHE["nc"]


def _perm_rows(w, bias_unused=None):
    """[C, D] -> [128, 4, D]: block (c,s) partition p = row 256c+2p+s."""
    D = w.shape[1]
    out = np.zeros((128, 4, D), np.float32)
    for c in range(2):
        for s in range(2):
            rows = 256 * c + 2 * np.arange(128) + s
            valid = rows < C
            out[valid, 2 * c + s, :] = w[rows[valid], :]
    return out


# ---------------------------------------------------------------- kernel --
def kernel(x, norm1_scale, norm1_bias, Wu, bu, Wg, bg, A,
           norm2_scale, norm2_bias, mlp_w1, mlp_b1, mlp_w2, mlp_b2,
           _return_raw=False):
    import ml_dtypes
    E4 = ml_dtypes.float8_e4m3
    f = np.float32
    x = np.asarray(x, f)
    norm1_scale = np.asarray(norm1_scale, f)
    norm1_bias = np.asarray(norm1_bias, f)
    Wu, bu = np.asarray(Wu, f), np.asarray(bu, f)
    Wg, bg = np.asarray(Wg, f), np.asarray(bg, f)
    A = np.asarray(A, f)
    norm2_scale = np.asarray(norm2_scale, f)
    norm2_bias = np.asarray(norm2_bias, f)
    mlp_w1, mlp_b1 = np.asarray(mlp_w1, f), np.asarray(mlp_b1, f)
    mlp_w2, mlp_b2 = np.asarray(mlp_w2, f), np.asarray(mlp_b2, f)

    # fold LN affine into downstream weights
    wu = norm1_scale[:, None] * Wu
    bu_f = bu + norm1_bias @ Wu
    wg = norm1_scale[:, None] * Wg
    bg_f = bg + norm1_bias @ Wg
    w1 = norm2_scale[:, None] * mlp_w1
    b1_f = mlp_b1 + norm2_bias @ mlp_w1

    wu_p = (_perm_rows(wu) * WS).astype(E4)
    wg_p = (_perm_rows(wg) * WS).astype(E4)
    w1_p = (_perm_rows(w1) * WS).astype(E4)

    a_p = np.zeros((128, 6, C), f)
    for s in range(6):
        a_p[:, s, :] = A[(s % 3) * 128:(s % 3) * 128 + 128, :]
    a_p = (a_p * WS).astype(E4)

    w2_p = np.zeros((128, MH, C), f)
    for s in range(MH):
        w2_p[:, s, :] = mlp_w2[s * 128:(s + 1) * 128, :]
    w2_p = (w2_p * WS).astype(E4)

    b2_p = (mlp_b2[None, :] * WS).astype(E4)
    ones_p = np.ones((1, 128), f).astype(E4)
    bu_p = np.ascontiguousarray((bu_f * WS).reshape(KT, 128).T)
    bg_p = np.ascontiguousarray((bg_f * 0.5).reshape(KT, 128).T)
    b1_p = np.ascontiguousarray(b1_f.reshape(MH, 128).T)

    cvals = np.array([EPS,
                      np.int32(1).view(f),
                      np.int32(MAGIC).view(f),
                      -0.5, 1.5, -1.0, 1.0 / C, 0.25, 0.5], f)
    cst_p = np.broadcast_to(cvals[None, :, None, None],
                            (128, 9, TPG, 1)).astype(f).copy()

    xs = x.reshape(NCORES, NTOK, C)
    in_maps = [{
        "x": np.ascontiguousarray(xs[i]),
        "wu": wu_p, "wg": wg_p, "a": a_p, "w1": w1_p, "w2": w2_p,
        "ones": ones_p, "b2": b2_p, "bu": bu_p, "bg": bg_p, "b1": b1_p,
        "cst": cst_p,
    } for i in range(NCORES)]

    res = run_bass_kernel_spmd(_get_nc(), in_maps, list(range(NCORES)))
    if _return_raw:
        return res
    out = np.concatenate([res.results[i]["out"] for i in range(NCORES)],
                         axis=0)
    return out.reshape(B, H, W, C).astype(np.float32)
